# revision 11
# baseline (speedup 1.0000x reference)
"""Trainium2 Bass kernel for MIGAttention (topk token masking + GQA attention).

Shapes (hardcoded): B=4, N=2048, C=1024, H=16 heads, HKV=4 kv-heads, DH=64,
keep-ratio 0.7 -> k = 1433 selected tokens per batch row.

Sharding: 8 cores = (batch b in 0..3) x (query-half h in 0..1).  Each core
receives x[b].T with token columns rolled by h*1024 so that its own query
half always occupies columns 0..1023 -> a single SPMD program for all cores.
Each core computes the full gate+topk mask and K/V for all 2048 tokens of its
batch, and attention + output projection for its 1024 queries.

Key compaction: masked tokens have k=v=0, so they contribute exactly
exp(0)=1 to every softmax denominator and 0 to every numerator.  After the
K|V projection the kernel compacts the 1433 selected tokens (padded to 1536
with an all-zero row) via gpsimd sparse_gather (index build) + dma_gather
(K transposed, V row-major with an embedded ones column), runs attention
over 12 key chunks instead of 16, and adds the constant 615 = N - KSEL to
the denominator to stand in for the dropped exp(0) terms.
"""

import contextlib
import sys

import numpy as np

if "/opt/trn_rl_repo" not in sys.path:
    sys.path.insert(0, "/opt/trn_rl_repo")

import concourse.bass as bass  # noqa: F401
import concourse.bass_isa as bass_isa
import concourse.mybir as mybir
from concourse import bacc
from concourse.tile import TileContext

F32 = mybir.dt.float32
F32R = mybir.dt.float32r
BF16 = mybir.dt.bfloat16
I16 = mybir.dt.int16
I32 = mybir.dt.int32
U32 = mybir.dt.uint32
AF = mybir.ActivationFunctionType
ALU = mybir.AluOpType

B, N, C = 4, 2048, 1024
H, HKV, DH = 16, 4, 64
NQ = N // 2          # queries per core
KSEL = 1433          # max(1, int(N * 0.7))
NSEL = 1536          # gathered key slots (KSEL padded up to mult of 128)
SENT = N             # sentinel index -> all-zero row in the K/V dram buffers
DENOM_C = float(N - KSEL)  # masked keys each add exp(0)=1 to the denominator
CC = C // 128        # contraction chunks (8)
KC = N // 128        # token chunks (16)
KCG = NSEL // 128    # gathered key chunks (12)
QT_D = H * DH        # 1024
KV_D = HKV * DH      # 256
N_ROUNDS = 5         # topk threshold refinement rounds
LO0, W0 = -4.0, 8.0  # initial logit search interval (logit std ~0.65)


def _emit(nc, tc, ctx, io):
    xT, wq, wkv, rw, wo, out_d = (
        io["xT"], io["wq"], io["wkv"], io["rw"], io["wo"], io["out"])

    # ---------------- long-lived pools ----------------
    # tile pools must close in LIFO order, so the open order below is the
    # exact reverse of the close order: pa (router scratch) closes first,
    # then pkv (after the gathers), pq (after QT), pm, px.
    const = ctx.enter_context(tc.tile_pool(name="const", bufs=1))
    small = ctx.enter_context(tc.tile_pool(name="small", bufs=1))
    big = ctx.enter_context(tc.tile_pool(name="big", bufs=1))
    dram = ctx.enter_context(tc.tile_pool(name="dram", bufs=1, space="DRAM"))

    px_ctx = contextlib.ExitStack()   # xT (alive through all projections)
    pm_ctx = contextlib.ExitStack()   # m_rep
    pq_ctx = contextlib.ExitStack()   # wq (Q projection)
    pkv_ctx = contextlib.ExitStack()  # wkv + kv staging
    pa_ctx = contextlib.ExitStack()   # router/refinement scratch
    px = px_ctx.enter_context(tc.tile_pool(name="px", bufs=1))
    psum1 = px_ctx.enter_context(tc.tile_pool(name="psum1", bufs=3, space="PSUM"))
    pm = pm_ctx.enter_context(tc.tile_pool(name="pm", bufs=1))
    pq = pq_ctx.enter_context(tc.tile_pool(name="pq", bufs=1))
    pkv = pkv_ctx.enter_context(tc.tile_pool(name="pkv", bufs=1))
    pa = pa_ctx.enter_context(tc.tile_pool(name="pa", bufs=1))
    psum_r = pa_ctx.enter_context(tc.tile_pool(name="psum_r", bufs=1, space="PSUM"))

    # ---------------- constants ----------------
    ones_row = const.tile([1, 128], F32)
    nc.vector.memset(ones_row, 1.0)
    iota128_i = const.tile([128, 1], I32)
    nc.gpsimd.iota(iota128_i, pattern=[[0, 1]], base=1, channel_multiplier=1)
    iota128 = const.tile([128, 1], F32)
    nc.vector.tensor_copy(iota128, iota128_i)
    # iota16p1[r, c] = 16*c + r + 1  (token id + 1 in the gpsimd [16, F] wrap)
    iota16_i = const.tile([16, N // 16], I32)
    nc.gpsimd.iota(iota16_i, pattern=[[16, N // 16]], base=1,
                   channel_multiplier=1)
    iota16p1 = const.tile([16, N // 16], F32)
    nc.vector.tensor_copy(iota16p1, iota16_i)
    # sel8[:, dd, p] = one-hot of the head owning oT partition p in d-chunk
    # dd; lhsT for denominator-row -> 128-partition broadcast matmuls
    sel8 = const.tile([16, CC, 128], F32R)
    nc.sync.dma_start(sel8, io["sel8"].bitcast(F32R))

    # ---------------- router first: logits = x @ rw (full fp32 for exact
    # topk order).  The streamed fp32 x chunks are then converted to the
    # bf16 xT_sb used by the Q and K|V projections (single HBM pass over x).
    rw_sb = pa.tile([128, CC], F32)
    for cc in range(CC):
        sl = slice(cc * 128, (cc + 1) * 128)
        nc.sync.dma_start(rw_sb[:, cc:cc + 1], rw[sl, :])
    # early weight loads (overlap the router/refinement chain)
    wq_sb = pq.tile([128, CC, QT_D], BF16)
    for cc in range(CC):
        nc.sync.dma_start(wq_sb[:, cc, :], wq[cc * 128:(cc + 1) * 128, :])
    wkv_sb = pkv.tile([128, CC, 2 * KV_D], BF16)
    for cc in range(CC):
        sl = slice(cc * 128, (cc + 1) * 128)
        nc.sync.dma_start(wkv_sb[:, cc, :], wkv[sl, :])

    xT_sb = px.tile([128, CC, N], BF16)
    logits_sb = pa.tile([1, N], F32)
    xr_pool = pa_ctx.enter_context(tc.tile_pool(name="xr_pool", bufs=2))
    rps = [psum_r.tile([1, 512], F32, tag=f"router_ps{g}", name=f"router_ps{g}")
           for g in range(4)]
    for cc in range(CC):
        xr = xr_pool.tile([128, N], F32, tag="xr", name=f"xr{cc}")
        nc.sync.dma_start(xr, xT[cc * 128:(cc + 1) * 128, :])
        for g in range(4):
            nc.tensor.matmul(
                rps[g], rw_sb[:, cc:cc + 1], xr[:, g * 512:(g + 1) * 512],
                start=(cc == 0), stop=(cc == CC - 1))
        # bf16 conversion for the projections, alternating engines so the
        # refinement chain (Scalar+Vector) is not starved by either queue
        if cc % 2 == 0:
            nc.scalar.copy(xT_sb[:, cc, :], xr)
        else:
            nc.vector.tensor_copy(xT_sb[:, cc, :], xr)
    for g in range(4):
        nc.vector.tensor_copy(logits_sb[:, g * 512:(g + 1) * 512], rps[g])

    # replicate logits across all 128 partitions (K=1 matmul broadcast)
    lrep = pa.tile([128, N], F32)
    for g in range(4):
        ps = psum_r.tile([128, 512], F32, tag="bcast_ps")
        nc.tensor.matmul(ps, ones_row, logits_sb[:, g * 512:(g + 1) * 512],
                         start=True, stop=True)
        nc.vector.tensor_copy(lrep[:, g * 512:(g + 1) * 512], ps)

    # ---------------- topk threshold refinement ----------------
    # invariant: v* (the KSEL-th largest logit) is in (lo, lo + w]
    lo = small.tile([128, 1], F32)
    nc.vector.memset(lo, LO0)
    neg_edges = small.tile([128, 1], F32)
    acc = small.tile([128, 1], F32)
    sel = small.tile([128, 1], F32)
    ssum = small.tile([128, 1], F32)
    sign_scr = pa.tile([128, N], BF16)  # Sign output is never read
    thr_acc = float(2 * KSEL - N)  # acc = #gt - #lt ; acc>=thr <=> #gt>=KSEL
    for r in range(N_ROUNDS):
        wstep = W0 / (128.0 ** (r + 1))
        # neg_edges[p] = -((p+1)*wstep + lo)  computed as iota*(-wstep) - lo
        nc.vector.scalar_tensor_tensor(
            neg_edges, iota128, -wstep, lo, op0=ALU.mult, op1=ALU.subtract)
        nc.scalar.activation(sign_scr, lrep, AF.Sign, bias=neg_edges,
                             scale=1.0, accum_out=acc)
        nc.vector.tensor_single_scalar(sel, acc, thr_acc, op=ALU.is_ge)
        nc.gpsimd.partition_all_reduce(ssum, sel, channels=128,
                                       reduce_op=bass_isa.ReduceOp.add)
        # lo += ssum * wstep   (bit-identical to the edge it selects)
        nc.vector.scalar_tensor_tensor(
            lo, ssum, wstep, lo, op0=ALU.mult, op1=ALU.add)

    # m = (logit > lo) * sigmoid(logit)   per token, replicated on partitions
    grep = pa.tile([128, N], F32)
    nc.scalar.activation(grep, lrep, AF.Sigmoid)
    m_rep = pm.tile([128, N], F32)
    nc.vector.scalar_tensor_tensor(
        m_rep, lrep, lo, grep, op0=ALU.is_gt, op1=ALU.mult)

    # m in token-major layout for K/V row scaling: m_v[p, i] = m[i*128 + p]
    m_dram = dram.tile([N], F32)
    nc.sync.dma_start(m_dram, m_rep[0:1, :])
    m_v = small.tile([128, KC], F32)
    nc.sync.dma_start(m_v, m_dram.rearrange("(i p) -> p i", p=128))

    # ---------------- compacted key index list ----------------
    # sel16[r, c] = token id (16c + r) if selected (m > 0) else -1, in the
    # gpsimd [16, F] free-major wrap; 8 trailing cols hold 128 sentinel
    # entries with id N, whose K/V dram row is all zeros.  sparse_gather
    # drops the negatives: entries 0..KSEL-1 = selected token ids (ascending)
    # followed by the sentinels; the first NSEL entries become the gather
    # index list (KSEL real + 103 zero-row pads).
    m16 = small.tile([16, N // 16 + 8], F32)
    nc.sync.dma_start(m16[:, 0:N // 16], m_dram.rearrange("(c r) -> r c", r=16))
    sel16 = small.tile([16, N // 16 + 8], F32)
    nc.vector.tensor_single_scalar(m16[:, 0:N // 16], m16[:, 0:N // 16], 0.0,
                                   op=ALU.is_gt)
    nc.vector.tensor_tensor(sel16[:, 0:N // 16], m16[:, 0:N // 16], iota16p1,
                            op=ALU.mult)
    nc.vector.tensor_single_scalar(sel16[:, 0:N // 16], sel16[:, 0:N // 16],
                                   -1.0, op=ALU.add)
    nc.vector.memset(sel16[:, N // 16:], float(SENT))
    # output is oversized (1664 slots) so the compaction never overflows it;
    # only the first NSEL entries (cols 0..95) are consumed.
    idxf = small.tile([16, 104], F32)
    nfound = small.tile([1, 1], U32)
    nc.gpsimd.sparse_gather(idxf, sel16, num_found=nfound)
    idx16 = small.tile([16, NSEL // 16], I16)
    nc.vector.tensor_copy(idx16, idxf[:, 0:NSEL // 16])
    # replicate to all 8 gpsimd cores' 16-partition windows
    idx128 = small.tile([128, NSEL // 16], I16)
    for k in range(8):
        nc.sync.dma_start(idx128[16 * k:16 * (k + 1), :], idx16)
    pa_ctx.close()

    # ---------------- K|V projection for all 2048 tokens ----------------
    # kv_sb row t (bf16): [ K: 4 kv-heads x 64 | V: 4 kv-heads x 64 ], scaled
    # by m[t].  Written to dram as gather sources:
    #   kdr [N+1, 256]  K rows, row N = 0
    #   vdr [N+1, 512]  V rows as per-kv [64 dims | 1 | 0*63], row N = 0
    # (the embedded ones column yields the softmax denominator via the same
    # att@V matmul; the zero row makes pad slots contribute nothing)
    kdr = dram.tile([N + 1, KV_D], BF16)
    vdr = dram.tile([N + 1, 4 * 128], BF16)
    kv_sb = pkv.tile([128, KC, 2 * KV_D], BF16)
    for i in range(KC):
        ps = psum1.tile([128, 512], F32, tag="proj_ps", name=f"kv_ps{i}")
        for cc in range(CC):
            nc.tensor.matmul(
                ps, xT_sb[:, cc, i * 128:(i + 1) * 128],
                wkv_sb[:, cc, :],
                start=(cc == 0), stop=(cc == CC - 1))
        nc.vector.tensor_scalar(
            kv_sb[:, i, :], ps, m_v[:, i:i + 1], None, op0=ALU.mult)
    nc.sync.dma_start(kdr[0:N, :].rearrange("(i p) d -> p i d", p=128),
                      kv_sb[:, :, 0:KV_D])
    # V 64-dim blocks spread to stride-128 slots; [ones, 0*63] tail per slot
    ones64 = pkv.tile([128, KC, 64], BF16)
    nc.vector.memset(ones64, 0.0)
    nc.vector.memset(ones64[:, :, 0:1], 1.0)
    vdr_v = vdr[0:N, :].rearrange("(i p) (g c) -> p i g c", p=128, c=128)
    for g in range(HKV):
        nc.sync.dma_start(vdr_v[:, :, g, 0:64],
                          kv_sb[:, :, KV_D + 64 * g:KV_D + 64 * (g + 1)])
        nc.sync.dma_start(vdr_v[:, :, g, 64:128], ones64)
    zrow = pkv.tile([1, 4 * 128], BF16)
    nc.vector.memset(zrow, 0.0)
    nc.sync.dma_start(kdr[N:N + 1, :], zrow[:, 0:KV_D])
    nc.sync.dma_start(vdr[N:N + 1, :], zrow)

    # ---------------- gather the selected K / V rows ----------------
    # chunked: the SWDGE descriptor ring holds ~1024 descriptors and a
    # single self-triggered dma_gather cannot reclaim its own entries, so
    # 1536-index calls crash the device.  Two 768-index calls each.
    GH = NSEL // 2  # 768
    kt_gs = [big.tile([128, 2, GH], BF16, name=f"kt_g{i}") for i in range(2)]
    v_sb = big.tile([128, KCG, HKV, 128], BF16)
    v_flat = v_sb.rearrange("p i g c -> p i (g c)")
    for i in range(2):
        isl = idx128[:, i * (GH // 16):(i + 1) * (GH // 16)]
        nc.gpsimd.dma_gather(kt_gs[i], kdr, isl, GH, GH, KV_D, transpose=True)
        nc.gpsimd.dma_gather(
            v_flat[:, i * (GH // 128):(i + 1) * (GH // 128), :], vdr, isl,
            GH, GH, 4 * 128, transpose=False)

    # ---------------- QT projection (overlaps the gather DMAs) ----------
    # QT[d, q] for my 1024 queries (columns 0..1023 of the rolled xT).
    # Slot layout is permuted so each q-head lands on the same partition range
    # as its GQA kv-head in KT: head h -> slot (h%4)+4*(h//8), partition base
    # ((h//4)%2)*64.  Slot j therefore holds heads (ha, ha+4), ha = j if j<4
    # else j+4, and wq columns are picked per head via a stride-4 head view.
    qt_sb = big.tile([128, H // 2, NQ], BF16)
    for j in range(H // 2):
        for g in range(NQ // 512):
            ps = psum1.tile([128, 512], F32, tag="proj_ps",
                            name=f"q_ps{j}_{g}")
            qs = slice(g * 512, (g + 1) * 512)
            for cc in range(CC):
                nc.tensor.matmul(
                    ps, wq_sb[:, cc, j * 128:(j + 1) * 128],
                    xT_sb[:, cc, qs],
                    start=(cc == 0), stop=(cc == CC - 1))
            nc.vector.tensor_tensor(qt_sb[:, j, qs], ps, m_rep[:, qs], op=ALU.mult)

    # KT with zeroed partner halves (emitted after QT so the gather wait does
    # not stall the QT evictions on the in-order DVE queue): kt_z[:, hkv] =
    # K of kv-head hkv on its own 64 partitions, zeros on the other 64 ->
    # the logits matmuls contract a full K=128 (the partner q-head's QT rows
    # hit zeros; full-array matmuls keep the PE HAM activity monitor warm).
    kt_z = big.tile([128, HKV, NSEL], BF16)
    nc.vector.memset(kt_z, 0.0)
    kt_zv = kt_z.rearrange("p (j two) n -> p two j n", two=2)
    for i in range(2):
        nsl = slice(i * GH, (i + 1) * GH)
        nc.vector.tensor_copy(kt_zv[0:64, 0, :, nsl], kt_gs[i][0:64, :, :])
        nc.vector.tensor_copy(kt_zv[64:128, 1, :, nsl], kt_gs[i][64:128, :, :])
    pkv_ctx.close()
    pq_ctx.close()
    pm_ctx.close()
    px_ctx.close()  # free xT + phase-1 PSUM

    # ---------------- phase 2: attention over 1536 gathered keys ----------
    ph2_ctx = contextlib.ExitStack()
    ph2 = ph2_ctx.enter_context(tc.tile_pool(name="ph2", bufs=1))
    wo_sb = ph2.tile([128, CC, C], F32R)
    for cc in range(CC):
        nc.sync.dma_start(wo_sb[:, cc, :],
                          wo[cc * 128:(cc + 1) * 128, :].bitcast(F32R))

    patt_ctx = contextlib.ExitStack()
    scr_pool = patt_ctx.enter_context(tc.tile_pool(name="scr_pool", bufs=2))
    p_pool = patt_ctx.enter_context(tc.tile_pool(name="p_pool", bufs=2))
    lg_pool = patt_ctx.enter_context(
        tc.tile_pool(name="lg_pool", bufs=2, space="PSUM"))
    att_pool = patt_ctx.enter_context(
        tc.tile_pool(name="att_pool", bufs=1, space="PSUM"))
    oT_sb = ph2.tile([128, CC, NQ], F32R)
    denom_sb = ph2.tile([16, NQ], F32)

    inv_sqrt_dh = float(1.0 / np.sqrt(DH))
    KQ = 2  # key chunks per P buffer
    # Head pairs (ha, ha+4) sit on disjoint partition halves (row-packed PE).
    pair_heads = [(ha, ha + 4) for ha in (0, 1, 2, 3, 8, 9, 10, 11)]
    for hp, pair in enumerate(pair_heads):
        att_ps = [att_pool.tile([128, NQ], F32, tag=f"att{m}", name=f"att{hp}_{m}")
                  for m in range(2)]
        pend = []  # pipelined attv matmuls: emitted one kc behind logits/exp
        for quarter in range(KCG // KQ):
            p_t = p_pool.tile([128, KQ, N], BF16, tag="p_t", name=f"p_{hp}_{quarter}")
            for kci in range(KQ):
                kc = quarter * KQ + kci
                lg = [lg_pool.tile([128, NQ], F32, tag="lg",
                                   name=f"lg{hp}_{kc}_{m2}") for m2 in range(2)]
                for m in range(2):
                    h = pair[m]
                    hkv = h // 4
                    jq = (h % 4) + 4 * (h // 8)
                    for g in range(NQ // 512):
                        nc.tensor.matmul(
                            lg[m][:, g * 512:(g + 1) * 512],
                            kt_z[:, hkv, kc * 128:(kc + 1) * 128],
                            qt_sb[:, jq, g * 512:(g + 1) * 512],
                            start=True, stop=True)
                for m in range(2):
                    nc.scalar.activation(
                        p_t[:, kci, m * NQ:(m + 1) * NQ], lg[m], AF.Exp,
                        scale=inv_sqrt_dh)
                # emit previous kc's attv matmuls now (keeps PE streaming)
                for f in pend:
                    f()
                pend = []

                def attv(p_t=p_t, kci=kci, kc=kc):
                    for m in range(2):
                        hk = pair[m] // 4
                        for g in range(NQ // 512):
                            nc.tensor.matmul(
                                att_ps[m][:, g * 512:(g + 1) * 512],
                                v_sb[:, kc, hk, :],
                                p_t[:, kci,
                                    m * NQ + g * 512:m * NQ + (g + 1) * 512],
                                start=(kc == 0), stop=(kc == KCG - 1))

                pend.append(attv)
        for f in pend:
            f()
        # fast evict (releases att psum quickly so PE stays HAM-warm):
        # copy [65, NQ] psum -> sbuf scratch, stash denom row, numerator
        # into oT unscaled; the 1/denom scale happens once after all pairs.
        for m in range(2):
            h = pair[m]
            scr65 = scr_pool.tile([65, NQ], F32R, tag="scr65",
                                  name=f"scr65_{hp}_{m}")
            nc.vector.tensor_copy(scr65, att_ps[m][0:65, :])
            nc.sync.dma_start(denom_sb[h:h + 1, :],
                              scr65[64:65, :].bitcast(F32))
            if h % 2 == 0:
                nc.vector.tensor_copy(oT_sb[0:64, h // 2, :], scr65[0:64, :])
            else:
                # partition shift 0 -> 64 must go through DMA
                nc.sync.dma_start(oT_sb[64:128, h // 2, :], scr65[0:64, :])

    # scale oT rows by 1/(denom + 615): one batched reciprocal (16 lanes),
    # then per-d-chunk broadcast of the two relevant denom rows via a tiny
    # sel8 matmul, and an in-place DVE multiply.  The +615 restores the
    # masked keys' exp(0)=1 terms dropped by the compaction.
    rec16 = ph2.tile([16, NQ], F32R)
    rec16_f = ph2.tile([16, NQ], F32)
    nc.vector.tensor_single_scalar(denom_sb, denom_sb, DENOM_C, op=ALU.add)
    with nc.allow_low_precision(reason="2e-5 rel err << output tolerance"):
        nc.vector.reciprocal_approx_fast(out=rec16_f, in_=denom_sb)
    nc.vector.tensor_copy(rec16, rec16_f)
    for dd in range(CC):
        for g in range(NQ // 512):
            bps = lg_pool.tile([128, 512], F32, tag="lg", name=f"bps{dd}_{g}")
            nc.tensor.matmul(
                bps, sel8[:, dd, :], rec16[:, g * 512:(g + 1) * 512],
                start=True, stop=True)
            sl = slice(g * 512, (g + 1) * 512)
            nc.vector.tensor_tensor(
                oT_sb[:, dd, sl], oT_sb[:, dd, sl], bps, op=ALU.mult)
    patt_ctx.close()
    # ---------------- phase 3: output projection ----------------
    ph3_ctx = contextlib.ExitStack()
    psum3 = ph3_ctx.enter_context(tc.tile_pool(name="psum3", bufs=4, space="PSUM"))
    out_pool = ph3_ctx.enter_context(tc.tile_pool(name="out_pool", bufs=2))
    for tt in range(NQ // 128):
        out_sb = out_pool.tile([128, C], F32, tag="out_sb", name=f"out_sb{tt}")
        for og in range(C // 512):
            ps = psum3.tile([128, 512], F32, tag="out_ps", name=f"out_ps{tt}_{og}")
            for dd in range(CC):
                nc.tensor.matmul(
                    ps, oT_sb[:, dd, tt * 128:(tt + 1) * 128],
                    wo_sb[:, dd, og * 512:(og + 1) * 512],
                    start=(dd == 0), stop=(dd == CC - 1))
            nc.scalar.copy(out_sb[:, og * 512:(og + 1) * 512], ps)
        nc.sync.dma_start(out_d[tt * 128:(tt + 1) * 128, :], out_sb)
    ph3_ctx.close()
    ph2_ctx.close()


_NC = None


def build_program():
    global _NC
    if _NC is not None:
        return _NC
    from contextlib import ExitStack

    nc = bacc.Bacc("TRN2", target_bir_lowering=False, debug=False, num_devices=8)
    io = {
        "xT": nc.dram_tensor("xT", (C, N), F32, kind="ExternalInput").ap(),
        "wq": nc.dram_tensor("wq", (C, QT_D), BF16, kind="ExternalInput").ap(),
        "wkv": nc.dram_tensor("wkv", (C, 2 * KV_D), BF16,
                              kind="ExternalInput").ap(),
        "rw": nc.dram_tensor("rw", (C, 1), F32, kind="ExternalInput").ap(),
        "wo": nc.dram_tensor("wo", (C, C), F32, kind="ExternalInput").ap(),
        "sel8": nc.dram_tensor("sel8", (16, CC, 128), F32,
                               kind="ExternalInput").ap(),
        "out": nc.dram_tensor("out", (NQ, C), F32, kind="ExternalOutput").ap(),
    }
    with TileContext(nc) as tc:
        with ExitStack() as ctx:
            _emit(nc, tc, ctx, io)
    nc.compile()
    _NC = nc
    return nc


def _permute_wq(wq):
    """Column-permute wq so QT slot j's 128 cols = heads (ha, ha+4) contig."""
    wq = np.asarray(wq, np.float32).reshape(C, H, DH)
    order = []
    for j in range(H // 2):
        ha = j if j < 4 else j + 4
        order += [ha, ha + 4]
    return np.ascontiguousarray(wq[:, order, :].reshape(C, H * DH))


def make_in_maps(x, router_w, wq, wk, wv, wo):
    import ml_dtypes

    bf16 = ml_dtypes.bfloat16
    wq = np.ascontiguousarray(_permute_wq(wq).astype(bf16))
    wkv = np.ascontiguousarray(np.concatenate(
        [np.asarray(wk, np.float32), np.asarray(wv, np.float32)],
        axis=1).astype(bf16))
    in_maps = []
    for core in range(8):
        b, h = core // 2, core % 2
        xT_core = np.ascontiguousarray(
            np.roll(np.asarray(x[b], np.float32).T, -h * NQ, axis=1))
        sel8 = np.zeros((16, CC, 128), np.float32)
        for dd in range(CC):
            for p in range(128):
                sel8[2 * dd + p // 64, dd, p] = 1.0
        in_maps.append({
            "xT": xT_core,
            "sel8": sel8,
            "wq": wq,
            "wkv": wkv,
            "rw": np.ascontiguousarray(router_w, dtype=np.float32),
            "wo": np.ascontiguousarray(wo, dtype=np.float32),
        })
    return in_maps


def _numpy_fallback(x, router_w, router_b, wq, bq, wk, bk, wv, bv, wo, bo):
    x = np.asarray(x, np.float32)
    gate = 1.0 / (1.0 + np.exp(-(x @ router_w + router_b)))
    xg = x * gate
    scores = gate[..., 0]
    idx = np.argsort(-scores, axis=-1, kind="stable")[:, :KSEL]
    mask = np.zeros((x.shape[0], x.shape[1]), np.float32)
    np.put_along_axis(mask, idx, 1.0, axis=1)
    xg = xg * mask[..., None]
    q = (xg @ wq + bq).reshape(B, N, H, DH)
    kk = np.repeat((xg @ wk + bk).reshape(B, N, HKV, DH), H // HKV, axis=2)
    v = np.repeat((xg @ wv + bv).reshape(B, N, HKV, DH), H // HKV, axis=2)
    att = np.einsum("bqhd,bkhd->bhqk", q, kk) / np.float32(np.sqrt(DH))
    att = att - att.max(-1, keepdims=True)
    att = np.exp(att)
    att = att / att.sum(-1, keepdims=True)
    o = np.einsum("bhqk,bkhd->bqhd", att, v).reshape(B, N, C)
    return (o @ wo + bo).astype(np.float32)


def kernel(x, router_w, router_b, wq, bq, wk, bk, wv, bv, wo, bo):
    x = np.asarray(x)
    biases = [router_b, bq, bk, bv, bo]
    if any(float(np.abs(np.asarray(t)).max()) != 0.0 for t in biases):
        # The device program folds away the (identically zero) biases; fall
        # back to an exact host implementation if that assumption breaks.
        return _numpy_fallback(x, router_w, router_b, wq, bq, wk, bk, wv, bv,
                               wo, bo)

    from concourse import bass_utils

    nc = build_program()
    in_maps = make_in_maps(x, router_w, wq, wk, wv, wo)
    res = bass_utils.run_bass_kernel_spmd(nc, in_maps, core_ids=list(range(8)))
    out = np.empty((B, N, C), np.float32)
    for core in range(8):
        b, h = core // 2, core % 2
        out[b, h * NQ:(h + 1) * NQ, :] = res.results[core]["out"]
    return out


# revision 17
# speedup vs baseline: 1.2409x; 1.2409x over previous
"""Trainium2 Bass kernel for MIGAttention (topk token masking + GQA attention).

Shapes (hardcoded): B=4, N=2048, C=1024, H=16 heads, HKV=4 kv-heads, DH=64,
keep-ratio 0.7 -> k = 1433 selected tokens per batch row.

Sharding: 8 cores = (batch b in 0..3) x (query-half h in 0..1).  Each core
receives x[b].T with token columns rolled by h*1024 so that its own query
half always occupies columns 0..1023 -> a single SPMD program for all cores.
Each core computes the full gate+topk mask and K/V for all 2048 tokens of its
batch, and attention + output projection for its 1024 queries.

Key compaction: masked tokens have k=v=0, so they contribute exactly
exp(0)=1 to every softmax denominator and 0 to every numerator.  After the
K|V projection the kernel compacts the 1433 selected tokens (padded to 1536
with an all-zero row) via gpsimd sparse_gather (index build) + dma_gather
(packed K|V rows), transposes K on the PE, and runs attention over 12 key
chunks instead of 16.  The gathered pad slots contribute exp(0)=1 each (zero
k row, memset ones column), so the denominator constant is 615-103=512.
"""

import contextlib
import sys

import numpy as np

if "/opt/trn_rl_repo" not in sys.path:
    sys.path.insert(0, "/opt/trn_rl_repo")

import concourse.bass as bass  # noqa: F401
import concourse.bass_isa as bass_isa
import concourse.mybir as mybir
from concourse import bacc
from concourse.tile import TileContext

F32 = mybir.dt.float32
F32R = mybir.dt.float32r
BF16 = mybir.dt.bfloat16
I16 = mybir.dt.int16
I32 = mybir.dt.int32
U32 = mybir.dt.uint32
AF = mybir.ActivationFunctionType
ALU = mybir.AluOpType

B, N, C = 4, 2048, 1024
H, HKV, DH = 16, 4, 64
NQ = N // 2          # queries per core
KSEL = 1433          # max(1, int(N * 0.7))
NSEL = 1536          # gathered key slots (KSEL padded up to mult of 128)
SENT = N             # sentinel index -> all-zero row in the K|V dram buffer
# masked keys each add exp(0)=1 to the softmax denominator; the NSEL-KSEL
# gathered pad slots already contribute theirs (zero k, ones col set)
DENOM_C = float(N - NSEL)
CC = C // 128        # contraction chunks (8)
KC = N // 128        # token chunks (16)
KCG = NSEL // 128    # gathered key chunks (12)
GH = NSEL // 2       # idx per dma_gather call (SWDGE ring holds ~1024 descs)
QT_D = H * DH        # 1024
KV_D = HKV * DH      # 256
N_ROUNDS = 5         # topk threshold refinement rounds
LO0, W0 = -4.0, 8.0  # initial logit search interval (logit std ~0.65)


def _emit(nc, tc, ctx, io):
    xT, wq, wkv, rw, wo, out_d = (
        io["xT"], io["wq"], io["wkv"], io["rw"], io["wo"], io["out"])

    # ---------------- long-lived pools ----------------
    # tile pools must close in LIFO order; open order is the exact reverse
    # of close order: psum_r (router psum) closes first, then psum_tr (K
    # transpose psum), pa (router scratch), pkv, pq, pm, px.
    const = ctx.enter_context(tc.tile_pool(name="const", bufs=1))
    small = ctx.enter_context(tc.tile_pool(name="small", bufs=1))
    big = ctx.enter_context(tc.tile_pool(name="big", bufs=1))
    dram = ctx.enter_context(tc.tile_pool(name="dram", bufs=1, space="DRAM"))

    px_ctx = contextlib.ExitStack()   # xT (alive through all projections)
    pm_ctx = contextlib.ExitStack()   # m_rep
    pq_ctx = contextlib.ExitStack()   # wq (Q projection)
    pkv_ctx = contextlib.ExitStack()  # wkv + kv staging
    pa_ctx = contextlib.ExitStack()   # router/refinement scratch
    pr_ctx = contextlib.ExitStack()   # router psum
    ptr_ctx = contextlib.ExitStack()  # K-transpose psum
    px = px_ctx.enter_context(tc.tile_pool(name="px", bufs=1))
    psum1 = px_ctx.enter_context(tc.tile_pool(name="psum1", bufs=6, space="PSUM"))
    pm = pm_ctx.enter_context(tc.tile_pool(name="pm", bufs=1))
    pq = pq_ctx.enter_context(tc.tile_pool(name="pq", bufs=1))
    pkv = pkv_ctx.enter_context(tc.tile_pool(name="pkv", bufs=1))
    pa = pa_ctx.enter_context(tc.tile_pool(name="pa", bufs=1))
    psum_r = pr_ctx.enter_context(tc.tile_pool(name="psum_r", bufs=1, space="PSUM"))

    # ---------------- constants ----------------
    ones_row = const.tile([1, 128], F32)
    nc.vector.memset(ones_row, 1.0)
    iota128_i = const.tile([128, 1], I32)
    nc.gpsimd.iota(iota128_i, pattern=[[0, 1]], base=1, channel_multiplier=1)
    iota128 = const.tile([128, 1], F32)
    nc.vector.tensor_copy(iota128, iota128_i)
    # iota16p1[r, c] = 16*c + r + 1  (token id + 1 in the gpsimd [16, F] wrap)
    iota16_i = const.tile([16, N // 16], I32)
    nc.gpsimd.iota(iota16_i, pattern=[[16, N // 16]], base=1,
                   channel_multiplier=1)
    iota16p1 = const.tile([16, N // 16], F32)
    nc.vector.tensor_copy(iota16p1, iota16_i)
    # identity (bf16) for PE-transposing the gathered K rows
    idn = const.tile([128, 128], BF16)
    nc.sync.dma_start(idn, io["idn"])
    # sel8[:, dd, p] = one-hot of the head owning oT partition p in d-chunk
    # dd; lhsT for denominator-row -> 128-partition broadcast matmuls
    sel8 = const.tile([16, CC, 128], F32R)
    nc.sync.dma_start(sel8, io["sel8"].bitcast(F32R))

    # ---------------- router first: logits = x @ rw (full fp32 for exact
    # topk order).  The streamed fp32 x chunks are then converted to the
    # bf16 xT_sb used by the Q and K|V projections (single HBM pass over x).
    rw_sb = pa.tile([128, CC], F32)
    for cc in range(CC):
        sl = slice(cc * 128, (cc + 1) * 128)
        nc.sync.dma_start(rw_sb[:, cc:cc + 1], rw[sl, :])
    xT_sb = px.tile([128, CC, N], BF16)
    logits_sb = pa.tile([1, N], F32)
    xr_pool = pa_ctx.enter_context(tc.tile_pool(name="xr_pool", bufs=2))
    # the 4 query-group accumulators pack into 2 psum banks (rows 0 and 64)
    rps = [psum_r.tile([65, 512], F32, tag=f"router_ps{t}",
                       name=f"router_ps{t}") for t in range(2)]

    def rps_row(g):
        return rps[g // 2][(g % 2) * 64:(g % 2) * 64 + 1, :]

    for cc in range(CC):
        xr = xr_pool.tile([128, N], F32, tag="xr", name=f"xr{cc}")
        nc.sync.dma_start(xr, xT[cc * 128:(cc + 1) * 128, :])
        for g in range(4):
            nc.tensor.matmul(
                rps_row(g), rw_sb[:, cc:cc + 1],
                xr[:, g * 512:(g + 1) * 512],
                start=(cc == 0), stop=(cc == CC - 1))
        # bf16 conversion for the projections, alternating engines so the
        # refinement chain (Scalar+Vector) is not starved by either queue
        if cc % 2 == 0:
            nc.scalar.copy(xT_sb[:, cc, :], xr)
        else:
            nc.vector.tensor_copy(xT_sb[:, cc, :], xr)
    for g in range(4):
        nc.vector.tensor_copy(logits_sb[:, g * 512:(g + 1) * 512], rps_row(g))

    # weight loads next on the DMA queues (overlap refinement + projections)
    wq_sb = pq.tile([128, CC, QT_D], BF16)
    for cc in range(CC):
        nc.sync.dma_start(wq_sb[:, cc, :], wq[cc * 128:(cc + 1) * 128, :])
    wkv_sb = pkv.tile([128, CC, 2 * KV_D], BF16)
    for cc in range(CC):
        sl = slice(cc * 128, (cc + 1) * 128)
        nc.sync.dma_start(wkv_sb[:, cc, :], wkv[sl, :])

    # replicate logits across all 128 partitions (K=1 matmul broadcast)
    lrep = pa.tile([128, N], F32)
    for g in range(4):
        ps = psum1.tile([128, 512], F32, tag="proj_ps", name=f"bcast{g}")
        nc.tensor.matmul(ps, ones_row, logits_sb[:, g * 512:(g + 1) * 512],
                         start=True, stop=True)
        nc.vector.tensor_copy(lrep[:, g * 512:(g + 1) * 512], ps)
    pr_ctx.close()

    # ---------------- K|V projection matmuls (PE runs these while the
    # scalar/vector/gpsimd engines work through the refinement chain; only
    # the m-scaled evictions below wait for the threshold)
    kv_ps = [psum1.tile([128, 512], F32, tag="proj_ps", name=f"kv_ps{i}")
             for i in range(KC)]
    for i in range(KC):
        for cc in range(CC):
            nc.tensor.matmul(
                kv_ps[i], xT_sb[:, cc, i * 128:(i + 1) * 128],
                wkv_sb[:, cc, :],
                start=(cc == 0), stop=(cc == CC - 1))

    # ---------------- topk threshold refinement ----------------
    # invariant: v* (the KSEL-th largest logit) is in (lo, lo + w]
    lo = small.tile([128, 1], F32)
    nc.vector.memset(lo, LO0)
    neg_edges = small.tile([128, 1], F32)
    acc = small.tile([128, 1], F32)
    sel = small.tile([128, 1], F32)
    ssum = small.tile([128, 1], F32)
    sign_scr = pa.tile([128, N], BF16)  # Sign output is never read
    thr_acc = float(2 * KSEL - N)  # acc = #gt - #lt ; acc>=thr <=> #gt>=KSEL
    for r in range(N_ROUNDS):
        wstep = W0 / (128.0 ** (r + 1))
        # neg_edges[p] = -((p+1)*wstep + lo)  computed as iota*(-wstep) - lo
        nc.vector.scalar_tensor_tensor(
            neg_edges, iota128, -wstep, lo, op0=ALU.mult, op1=ALU.subtract)
        nc.scalar.activation(sign_scr, lrep, AF.Sign, bias=neg_edges,
                             scale=1.0, accum_out=acc)
        nc.vector.tensor_single_scalar(sel, acc, thr_acc, op=ALU.is_ge)
        nc.gpsimd.partition_all_reduce(ssum, sel, channels=128,
                                       reduce_op=bass_isa.ReduceOp.add)
        # lo += ssum * wstep   (bit-identical to the edge it selects)
        nc.vector.scalar_tensor_tensor(
            lo, ssum, wstep, lo, op0=ALU.mult, op1=ALU.add)

    # m = (logit > lo) * sigmoid(logit)   per token, replicated on partitions
    grep = pa.tile([128, N], F32)
    nc.scalar.activation(grep, lrep, AF.Sigmoid)
    m_rep = pm.tile([128, N], F32)
    nc.vector.scalar_tensor_tensor(
        m_rep, lrep, lo, grep, op0=ALU.is_gt, op1=ALU.mult)

    # m in token-major layout for K/V row scaling: m_v[p, i] = m[i*128 + p]
    m_dram = dram.tile([N], F32)
    nc.sync.dma_start(m_dram, m_rep[0:1, :])
    m_v = small.tile([128, KC], F32)
    nc.sync.dma_start(m_v, m_dram.rearrange("(i p) -> p i", p=128))

    # ---------------- compacted key index list ----------------
    # sel16[r, c] = token id (16c + r) if selected (m > 0) else -1, in the
    # gpsimd [16, F] free-major wrap; 8 trailing cols hold 128 sentinel
    # entries with id N, whose K|V dram row is all zeros.  sparse_gather
    # drops the negatives: entries 0..KSEL-1 = selected token ids (ascending)
    # followed by the sentinels; the first NSEL entries become the gather
    # index list (KSEL real + 103 zero-row pads).
    m16 = small.tile([16, N // 16 + 8], F32)
    nc.sync.dma_start(m16[:, 0:N // 16], m_dram.rearrange("(c r) -> r c", r=16))
    sel16 = small.tile([16, N // 16 + 8], F32)
    nc.vector.tensor_single_scalar(m16[:, 0:N // 16], m16[:, 0:N // 16], 0.0,
                                   op=ALU.is_gt)
    nc.vector.tensor_tensor(sel16[:, 0:N // 16], m16[:, 0:N // 16], iota16p1,
                            op=ALU.mult)
    nc.vector.tensor_single_scalar(sel16[:, 0:N // 16], sel16[:, 0:N // 16],
                                   -1.0, op=ALU.add)
    nc.vector.memset(sel16[:, N // 16:], float(SENT))
    # output is oversized (1664 slots) so the compaction never overflows it;
    # only the first NSEL entries (cols 0..95) are consumed.
    idxf = small.tile([16, 104], F32)
    nfound = small.tile([1, 1], U32)
    nc.gpsimd.sparse_gather(idxf, sel16, num_found=nfound)
    idx16 = small.tile([16, NSEL // 16], I16)
    nc.vector.tensor_copy(idx16, idxf[:, 0:NSEL // 16])
    # replicate to all 8 gpsimd cores' 16-partition windows
    idx128 = small.tile([128, NSEL // 16], I16)
    for k in range(8):
        nc.sync.dma_start(idx128[16 * k:16 * (k + 1), :], idx16)

    # ---------------- K|V eviction + dram staging ----------------
    # kv row t (bf16): [ K: 4 kv-heads x 64 | V: 4 kv-heads x 64 ], scaled by
    # m[t]; one contiguous write to kvdr [N+1, 512], row N kept all-zero.
    kvdr = dram.tile([N + 1, 2 * KV_D], BF16)
    kv_sb = pkv.tile([128, KC, 2 * KV_D], BF16)
    for i in range(KC):
        nc.vector.tensor_scalar(
            kv_sb[:, i, :], kv_ps[i], m_v[:, i:i + 1], None, op0=ALU.mult)
    nc.sync.dma_start(kvdr[0:N, :].rearrange("(i p) d -> p i d", p=128), kv_sb)
    zrow = pkv.tile([1, 2 * KV_D], BF16)
    nc.vector.memset(zrow, 0.0)
    nc.sync.dma_start(kvdr[N:N + 1, :], zrow)

    # ---------------- gather the selected K|V rows ----------------
    # chunked: the SWDGE descriptor ring holds ~1024 descriptors and a
    # single self-triggered dma_gather cannot reclaim its own entries, so
    # one 1536-index call would crash the device.  Two 768-index calls.
    kv_g = big.tile([128, KCG, 2 * KV_D], BF16)
    for i in range(2):
        isl = idx128[:, i * (GH // 16):(i + 1) * (GH // 16)]
        nc.gpsimd.dma_gather(
            kv_g[:, i * (GH // 128):(i + 1) * (GH // 128), :], kvdr, isl,
            GH, GH, 2 * KV_D, transpose=False)

    # ---------------- QT projection (overlaps the gather DMAs) ----------
    # QT[d, q] for my 1024 queries (columns 0..1023 of the rolled xT).
    # Slot layout is permuted so each q-head lands on the same partition range
    # as its GQA kv-head in KT: head h -> slot (h%4)+4*(h//8), partition base
    # ((h//4)%2)*64.  Slot j therefore holds heads (ha, ha+4), ha = j if j<4
    # else j+4, and wq columns are picked per head via a stride-4 head view.
    qt_sb = big.tile([128, H // 2, NQ], BF16)
    for j in range(H // 2):
        for g in range(NQ // 512):
            ps = psum1.tile([128, 512], F32, tag="proj_ps",
                            name=f"q_ps{j}_{g}")
            qs = slice(g * 512, (g + 1) * 512)
            for cc in range(CC):
                nc.tensor.matmul(
                    ps, wq_sb[:, cc, j * 128:(j + 1) * 128],
                    xT_sb[:, cc, qs],
                    start=(cc == 0), stop=(cc == CC - 1))
            nc.vector.tensor_tensor(qt_sb[:, j, qs], ps, m_rep[:, qs], op=ALU.mult)

    # ---------------- KT via PE transpose + V stationary build ----------
    # kt_z[:, hkv] = K of kv-head hkv on its own 64 partitions, zeros on the
    # other 64 -> the logits matmuls contract a full K=128 (the partner
    # q-head's QT rows hit zeros; full-array matmuls keep the PE HAM warm).
    psum_tr = ptr_ctx.enter_context(
        tc.tile_pool(name="psum_tr", bufs=2, space="PSUM"))
    kt_z = big.tile([128, HKV, NSEL], BF16)
    nc.vector.memset(kt_z, 0.0)
    for kc in range(KCG):
        for pairg in range(2):  # kv-heads (0,1) then (2,3)
            trp = psum_tr.tile([128, 128], BF16, tag="tr",
                               name=f"tr{kc}_{pairg}")
            nc.tensor.transpose(trp, kv_g[:, kc, pairg * 128:(pairg + 1) * 128],
                                idn)
            ksl = slice(kc * 128, (kc + 1) * 128)
            nc.vector.tensor_copy(kt_z[0:64, 2 * pairg, ksl], trp[0:64, :])
            nc.scalar.copy(kt_z[64:128, 2 * pairg + 1, ksl], trp[64:128, :])
    # v65[tok, kc, g, :] = [64 v dims | 1 | 0*63]; the ones column yields the
    # softmax denominator through the same att@V matmul (pad slots included:
    # their p=exp(0)=1 joins the masked keys' constant, hence DENOM_C=512).
    v65 = big.tile([128, KCG, HKV, 128], BF16)
    nc.vector.memset(v65, 0.0)
    nc.vector.tensor_copy(
        v65[:, :, :, 0:64],
        kv_g[:, :, KV_D:].rearrange("p i (g c) -> p i g c", c=64))
    nc.vector.memset(v65[:, :, :, 64:65], 1.0)
    ptr_ctx.close()
    pa_ctx.close()
    pkv_ctx.close()
    pq_ctx.close()
    pm_ctx.close()
    px_ctx.close()  # free xT + phase-1 PSUM

    # ---------------- phase 2: attention over 1536 gathered keys ----------
    ph2_ctx = contextlib.ExitStack()
    ph2 = ph2_ctx.enter_context(tc.tile_pool(name="ph2", bufs=1))
    wo_sb = ph2.tile([128, CC, C], F32R)
    for cc in range(CC):
        nc.sync.dma_start(wo_sb[:, cc, :],
                          wo[cc * 128:(cc + 1) * 128, :].bitcast(F32R))

    patt_ctx = contextlib.ExitStack()
    scr_pool = patt_ctx.enter_context(tc.tile_pool(name="scr_pool", bufs=2))
    p_pool = patt_ctx.enter_context(tc.tile_pool(name="p_pool", bufs=2))
    lg_pool = patt_ctx.enter_context(
        tc.tile_pool(name="lg_pool", bufs=2, space="PSUM"))
    att_pool = patt_ctx.enter_context(
        tc.tile_pool(name="att_pool", bufs=1, space="PSUM"))
    oT_sb = ph2.tile([128, CC, NQ], F32R)
    denom_sb = ph2.tile([16, NQ], F32)

    inv_sqrt_dh = float(1.0 / np.sqrt(DH))
    KQ = 2  # key chunks per P buffer
    # Head pairs (ha, ha+4) sit on disjoint partition halves (row-packed PE).
    pair_heads = [(ha, ha + 4) for ha in (0, 1, 2, 3, 8, 9, 10, 11)]
    for hp, pair in enumerate(pair_heads):
        att_ps = [att_pool.tile([128, NQ], F32, tag=f"att{m}", name=f"att{hp}_{m}")
                  for m in range(2)]
        pend = []  # pipelined attv matmuls: emitted one kc behind logits/exp
        for quarter in range(KCG // KQ):
            p_t = p_pool.tile([128, KQ, N], BF16, tag="p_t", name=f"p_{hp}_{quarter}")
            for kci in range(KQ):
                kc = quarter * KQ + kci
                lg = [lg_pool.tile([128, NQ], F32, tag="lg",
                                   name=f"lg{hp}_{kc}_{m2}") for m2 in range(2)]
                for m in range(2):
                    h = pair[m]
                    hkv = h // 4
                    jq = (h % 4) + 4 * (h // 8)
                    for g in range(NQ // 512):
                        nc.tensor.matmul(
                            lg[m][:, g * 512:(g + 1) * 512],
                            kt_z[:, hkv, kc * 128:(kc + 1) * 128],
                            qt_sb[:, jq, g * 512:(g + 1) * 512],
                            start=True, stop=True)
                for m in range(2):
                    nc.scalar.activation(
                        p_t[:, kci, m * NQ:(m + 1) * NQ], lg[m], AF.Exp,
                        scale=inv_sqrt_dh)
                # emit previous kc's attv matmuls now (keeps PE streaming)
                for f in pend:
                    f()
                pend = []

                def attv(p_t=p_t, kci=kci, kc=kc):
                    for m in range(2):
                        hk = pair[m] // 4
                        for g in range(NQ // 512):
                            nc.tensor.matmul(
                                att_ps[m][:, g * 512:(g + 1) * 512],
                                v65[:, kc, hk, :],
                                p_t[:, kci,
                                    m * NQ + g * 512:m * NQ + (g + 1) * 512],
                                start=(kc == 0), stop=(kc == KCG - 1))

                pend.append(attv)
        for f in pend:
            f()
        # fast evict (releases att psum quickly so PE stays HAM-warm):
        # copy [65, NQ] psum -> sbuf scratch, stash denom row, numerator
        # into oT unscaled; the 1/denom scale happens once after all pairs.
        for m in range(2):
            h = pair[m]
            scr65 = scr_pool.tile([65, NQ], F32R, tag="scr65",
                                  name=f"scr65_{hp}_{m}")
            nc.vector.tensor_copy(scr65, att_ps[m][0:65, :])
            nc.sync.dma_start(denom_sb[h:h + 1, :],
                              scr65[64:65, :].bitcast(F32))
            if h % 2 == 0:
                nc.vector.tensor_copy(oT_sb[0:64, h // 2, :], scr65[0:64, :])
            else:
                # partition shift 0 -> 64 must go through DMA
                nc.sync.dma_start(oT_sb[64:128, h // 2, :], scr65[0:64, :])

    # scale oT rows by 1/(denom + 512): one batched reciprocal (16 lanes),
    # then per-d-chunk broadcast of the two relevant denom rows via a tiny
    # sel8 matmul, and an in-place DVE multiply.  The +512 restores the
    # non-gathered masked keys' exp(0)=1 terms dropped by the compaction.
    rec16 = ph2.tile([16, NQ], F32R)
    rec16_f = ph2.tile([16, NQ], F32)
    nc.vector.tensor_single_scalar(denom_sb, denom_sb, DENOM_C, op=ALU.add)
    with nc.allow_low_precision(reason="2e-5 rel err << output tolerance"):
        nc.vector.reciprocal_approx_fast(out=rec16_f, in_=denom_sb)
    nc.vector.tensor_copy(rec16, rec16_f)
    for dd in range(CC):
        for g in range(NQ // 512):
            bps = lg_pool.tile([128, 512], F32, tag="lg", name=f"bps{dd}_{g}")
            nc.tensor.matmul(
                bps, sel8[:, dd, :], rec16[:, g * 512:(g + 1) * 512],
                start=True, stop=True)
            sl = slice(g * 512, (g + 1) * 512)
            nc.vector.tensor_tensor(
                oT_sb[:, dd, sl], oT_sb[:, dd, sl], bps, op=ALU.mult)
    patt_ctx.close()
    # ---------------- phase 3: output projection ----------------
    ph3_ctx = contextlib.ExitStack()
    psum3 = ph3_ctx.enter_context(tc.tile_pool(name="psum3", bufs=4, space="PSUM"))
    out_pool = ph3_ctx.enter_context(tc.tile_pool(name="out_pool", bufs=2))
    for tt in range(NQ // 128):
        out_sb = out_pool.tile([128, C], F32, tag="out_sb", name=f"out_sb{tt}")
        for og in range(C // 512):
            ps = psum3.tile([128, 512], F32, tag="out_ps", name=f"out_ps{tt}_{og}")
            for dd in range(CC):
                nc.tensor.matmul(
                    ps, oT_sb[:, dd, tt * 128:(tt + 1) * 128],
                    wo_sb[:, dd, og * 512:(og + 1) * 512],
                    start=(dd == 0), stop=(dd == CC - 1))
            nc.scalar.copy(out_sb[:, og * 512:(og + 1) * 512], ps)
        nc.sync.dma_start(out_d[tt * 128:(tt + 1) * 128, :], out_sb)
    ph3_ctx.close()
    ph2_ctx.close()


_NC = None


def build_program():
    global _NC
    if _NC is not None:
        return _NC
    from contextlib import ExitStack

    nc = bacc.Bacc("TRN2", target_bir_lowering=False, debug=False, num_devices=8)
    io = {
        "xT": nc.dram_tensor("xT", (C, N), F32, kind="ExternalInput").ap(),
        "wq": nc.dram_tensor("wq", (C, QT_D), BF16, kind="ExternalInput").ap(),
        "wkv": nc.dram_tensor("wkv", (C, 2 * KV_D), BF16,
                              kind="ExternalInput").ap(),
        "rw": nc.dram_tensor("rw", (C, 1), F32, kind="ExternalInput").ap(),
        "wo": nc.dram_tensor("wo", (C, C), F32, kind="ExternalInput").ap(),
        "sel8": nc.dram_tensor("sel8", (16, CC, 128), F32,
                               kind="ExternalInput").ap(),
        "idn": nc.dram_tensor("idn", (128, 128), BF16,
                              kind="ExternalInput").ap(),
        "out": nc.dram_tensor("out", (NQ, C), F32, kind="ExternalOutput").ap(),
    }
    with TileContext(nc) as tc:
        with ExitStack() as ctx:
            _emit(nc, tc, ctx, io)
    nc.compile()
    _NC = nc
    return nc


def _permute_wq(wq):
    """Column-permute wq so QT slot j's 128 cols = heads (ha, ha+4) contig."""
    wq = np.asarray(wq, np.float32).reshape(C, H, DH)
    order = []
    for j in range(H // 2):
        ha = j if j < 4 else j + 4
        order += [ha, ha + 4]
    return np.ascontiguousarray(wq[:, order, :].reshape(C, H * DH))


def make_in_maps(x, router_w, wq, wk, wv, wo):
    import ml_dtypes

    bf16 = ml_dtypes.bfloat16
    wq = np.ascontiguousarray(_permute_wq(wq).astype(bf16))
    wkv = np.ascontiguousarray(np.concatenate(
        [np.asarray(wk, np.float32), np.asarray(wv, np.float32)],
        axis=1).astype(bf16))
    in_maps = []
    for core in range(8):
        b, h = core // 2, core % 2
        xT_core = np.ascontiguousarray(
            np.roll(np.asarray(x[b], np.float32).T, -h * NQ, axis=1))
        sel8 = np.zeros((16, CC, 128), np.float32)
        for dd in range(CC):
            for p in range(128):
                sel8[2 * dd + p // 64, dd, p] = 1.0
        in_maps.append({
            "xT": xT_core,
            "sel8": sel8,
            "idn": np.eye(128, dtype=bf16),
            "wq": wq,
            "wkv": wkv,
            "rw": np.ascontiguousarray(router_w, dtype=np.float32),
            "wo": np.ascontiguousarray(wo, dtype=np.float32),
        })
    return in_maps


def _numpy_fallback(x, router_w, router_b, wq, bq, wk, bk, wv, bv, wo, bo):
    x = np.asarray(x, np.float32)
    gate = 1.0 / (1.0 + np.exp(-(x @ router_w + router_b)))
    xg = x * gate
    scores = gate[..., 0]
    idx = np.argsort(-scores, axis=-1, kind="stable")[:, :KSEL]
    mask = np.zeros((x.shape[0], x.shape[1]), np.float32)
    np.put_along_axis(mask, idx, 1.0, axis=1)
    xg = xg * mask[..., None]
    q = (xg @ wq + bq).reshape(B, N, H, DH)
    kk = np.repeat((xg @ wk + bk).reshape(B, N, HKV, DH), H // HKV, axis=2)
    v = np.repeat((xg @ wv + bv).reshape(B, N, HKV, DH), H // HKV, axis=2)
    att = np.einsum("bqhd,bkhd->bhqk", q, kk) / np.float32(np.sqrt(DH))
    att = att - att.max(-1, keepdims=True)
    att = np.exp(att)
    att = att / att.sum(-1, keepdims=True)
    o = np.einsum("bhqk,bkhd->bqhd", att, v).reshape(B, N, C)
    return (o @ wo + bo).astype(np.float32)


def kernel(x, router_w, router_b, wq, bq, wk, bk, wv, bv, wo, bo):
    x = np.asarray(x)
    biases = [router_b, bq, bk, bv, bo]
    if any(float(np.abs(np.asarray(t)).max()) != 0.0 for t in biases):
        # The device program folds away the (identically zero) biases; fall
        # back to an exact host implementation if that assumption breaks.
        return _numpy_fallback(x, router_w, router_b, wq, bq, wk, bk, wv, bv,
                               wo, bo)

    from concourse import bass_utils

    nc = build_program()
    in_maps = make_in_maps(x, router_w, wq, wk, wv, wo)
    res = bass_utils.run_bass_kernel_spmd(nc, in_maps, core_ids=list(range(8)))
    out = np.empty((B, N, C), np.float32)
    for core in range(8):
        b, h = core // 2, core % 2
        out[b, h * NQ:(h + 1) * NQ, :] = res.results[core]["out"]
    return out


# revision 23
# speedup vs baseline: 1.2731x; 1.0260x over previous
"""Trainium2 Bass kernel for MIGAttention (topk token masking + GQA attention).

Shapes (hardcoded): B=4, N=2048, C=1024, H=16 heads, HKV=4 kv-heads, DH=64,
keep-ratio 0.7 -> k = 1433 selected tokens per batch row.

Sharding: 8 cores = (batch b in 0..3) x (query-half h in 0..1).  Each core
receives x[b].T with token columns rolled by h*1024 so that its own query
half always occupies columns 0..1023 -> a single SPMD program for all cores.
Each core computes the full gate+topk mask and K/V for all 2048 tokens of its
batch, and attention + output projection for its 1024 queries.

Key compaction: masked tokens have k=v=0, so they contribute exactly
exp(0)=1 to every softmax denominator and 0 to every numerator.  After the
K|V projection the kernel compacts the 1433 selected tokens (padded to 1536
with an all-zero row) via gpsimd sparse_gather (index build) + dma_gather
(packed K|V rows), transposes K on the PE, and runs attention over 12 key
chunks instead of 16.  The gathered pad slots contribute exp(0)=1 each (zero
k row, memset ones column), so the denominator constant is 615-103=512.
"""

import contextlib
import sys

import numpy as np

if "/opt/trn_rl_repo" not in sys.path:
    sys.path.insert(0, "/opt/trn_rl_repo")

import concourse.bass as bass  # noqa: F401
import concourse.bass_isa as bass_isa
import concourse.mybir as mybir
from concourse import bacc
from concourse.tile import TileContext

F32 = mybir.dt.float32
F32R = mybir.dt.float32r
BF16 = mybir.dt.bfloat16
I16 = mybir.dt.int16
I32 = mybir.dt.int32
U32 = mybir.dt.uint32
AF = mybir.ActivationFunctionType
ALU = mybir.AluOpType

B, N, C = 4, 2048, 1024
H, HKV, DH = 16, 4, 64
NQ = N // 2          # queries per core
KSEL = 1433          # max(1, int(N * 0.7))
NSEL = 1536          # gathered key slots (KSEL padded up to mult of 128)
SENT = N             # sentinel index -> all-zero row in the K|V dram buffer
# masked keys each add exp(0)=1 to the softmax denominator; the NSEL-KSEL
# gathered pad slots already contribute theirs (zero k, ones col set)
DENOM_C = float(N - NSEL)
CC = C // 128        # contraction chunks (8)
KC = N // 128        # token chunks (16)
KCG = NSEL // 128    # gathered key chunks (12)
GH = NSEL // 2       # idx per dma_gather call (SWDGE ring holds ~1024 descs)
QT_D = H * DH        # 1024
KV_D = HKV * DH      # 256
N_ROUNDS = 4         # topk threshold refinement rounds (interval 8/128^4
                     # ~3e-8 wide; a logit landing inside is ~4e-5 unlikely,
                     # and an off-by-one selection costs ~0.05% rel err)
LO0, W0 = -4.0, 8.0  # initial logit search interval (logit std ~0.65)


def _emit(nc, tc, ctx, io):
    xT, wq, wkv, rw, wo, out_d = (
        io["xT"], io["wq"], io["wkv"], io["rw"], io["wo"], io["out"])

    # ---------------- long-lived pools ----------------
    # tile pools must close in LIFO order; open order is the exact reverse
    # of close order: psum_r (router psum) closes first, then psum_tr (K
    # transpose psum), pa (router scratch), pkv, pq, pm, px.
    const = ctx.enter_context(tc.tile_pool(name="const", bufs=1))
    small = ctx.enter_context(tc.tile_pool(name="small", bufs=1))
    big = ctx.enter_context(tc.tile_pool(name="big", bufs=1))
    dram = ctx.enter_context(tc.tile_pool(name="dram", bufs=1, space="DRAM"))

    px_ctx = contextlib.ExitStack()   # xT (alive through all projections)
    pm_ctx = contextlib.ExitStack()   # m_rep
    pq_ctx = contextlib.ExitStack()   # wq (Q projection)
    pkv_ctx = contextlib.ExitStack()  # wkv + kv staging
    pa_ctx = contextlib.ExitStack()   # router/refinement scratch
    pr_ctx = contextlib.ExitStack()   # router psum
    ptr_ctx = contextlib.ExitStack()  # K-transpose psum
    px = px_ctx.enter_context(tc.tile_pool(name="px", bufs=1))
    psum1 = px_ctx.enter_context(tc.tile_pool(name="psum1", bufs=6, space="PSUM"))
    pm = pm_ctx.enter_context(tc.tile_pool(name="pm", bufs=1))
    pq = pq_ctx.enter_context(tc.tile_pool(name="pq", bufs=1))
    pkv = pkv_ctx.enter_context(tc.tile_pool(name="pkv", bufs=1))
    pa = pa_ctx.enter_context(tc.tile_pool(name="pa", bufs=1))
    psum_r = pr_ctx.enter_context(tc.tile_pool(name="psum_r", bufs=1, space="PSUM"))

    # ---------------- constants ----------------
    ones_row = const.tile([1, 128], F32)
    nc.vector.memset(ones_row, 1.0)
    iota128_i = const.tile([128, 1], I32)
    nc.gpsimd.iota(iota128_i, pattern=[[0, 1]], base=1, channel_multiplier=1)
    iota128 = const.tile([128, 1], F32)
    nc.vector.tensor_copy(iota128, iota128_i)
    # iota16p1[r, c] = 16*c + r + 1  (token id + 1 in the gpsimd [16, F] wrap)
    iota16_i = const.tile([16, N // 16], I32)
    nc.gpsimd.iota(iota16_i, pattern=[[16, N // 16]], base=1,
                   channel_multiplier=1)
    iota16p1 = const.tile([16, N // 16], F32)
    nc.vector.tensor_copy(iota16p1, iota16_i)
    # identity (bf16) for PE-transposing the gathered K rows
    idn = const.tile([128, 128], BF16)
    nc.sync.dma_start(idn, io["idn"])
    # sel8[:, dd, p] = one-hot of the head owning oT partition p in d-chunk
    # dd; lhsT for denominator-row -> 128-partition broadcast matmuls
    sel8 = const.tile([16, CC, 128], F32R)
    nc.sync.dma_start(sel8, io["sel8"].bitcast(F32R))

    # ---------------- router first: logits = x @ rw (full fp32 for exact
    # topk order).  The streamed fp32 x chunks are then converted to the
    # bf16 xT_sb used by the Q and K|V projections (single HBM pass over x).
    rw_sb = pa.tile([128, CC], F32)
    for cc in range(CC):
        sl = slice(cc * 128, (cc + 1) * 128)
        nc.sync.dma_start(rw_sb[:, cc:cc + 1], rw[sl, :])
    xT_sb = px.tile([128, CC, N], BF16)
    logits_sb = pa.tile([1, N], F32)
    xr_pool = pa_ctx.enter_context(tc.tile_pool(name="xr_pool", bufs=2))
    # the 4 query-group accumulators pack into 2 psum banks (rows 0 and 64)
    rps = [psum_r.tile([65, 512], F32, tag=f"router_ps{t}",
                       name=f"router_ps{t}") for t in range(2)]

    def rps_row(g):
        return rps[g // 2][(g % 2) * 64:(g % 2) * 64 + 1, :]

    for cc in range(CC):
        xr = xr_pool.tile([128, N], F32, tag="xr", name=f"xr{cc}")
        nc.sync.dma_start(xr, xT[cc * 128:(cc + 1) * 128, :])
        for g in range(4):
            nc.tensor.matmul(
                rps_row(g), rw_sb[:, cc:cc + 1],
                xr[:, g * 512:(g + 1) * 512],
                start=(cc == 0), stop=(cc == CC - 1))
        # bf16 conversion for the projections, alternating engines so the
        # refinement chain (Scalar+Vector) is not starved by either queue
        if cc % 2 == 0:
            nc.scalar.copy(xT_sb[:, cc, :], xr)
        else:
            nc.vector.tensor_copy(xT_sb[:, cc, :], xr)
    for g in range(4):
        nc.vector.tensor_copy(logits_sb[:, g * 512:(g + 1) * 512], rps_row(g))

    # weight loads next on the DMA queues (overlap refinement + projections)
    wq_sb = pq.tile([128, CC, QT_D], BF16)
    for cc in range(CC):
        nc.sync.dma_start(wq_sb[:, cc, :], wq[cc * 128:(cc + 1) * 128, :])
    wkv_sb = pkv.tile([128, CC, 2 * KV_D], BF16)
    for cc in range(CC):
        sl = slice(cc * 128, (cc + 1) * 128)
        nc.sync.dma_start(wkv_sb[:, cc, :], wkv[sl, :])

    # replicate logits across all 128 partitions (K=1 matmul broadcast)
    lrep = pa.tile([128, N], F32)
    for g in range(4):
        ps = psum1.tile([128, 512], F32, tag="proj_ps", name=f"bcast{g}")
        nc.tensor.matmul(ps, ones_row, logits_sb[:, g * 512:(g + 1) * 512],
                         start=True, stop=True)
        nc.vector.tensor_copy(lrep[:, g * 512:(g + 1) * 512], ps)
    pr_ctx.close()

    # ---------------- K|V projection matmuls (PE runs these while the
    # scalar/vector/gpsimd engines work through the refinement chain; only
    # the m-scaled evictions below wait for the threshold)
    kv_ps = [psum1.tile([128, 512], F32, tag="proj_ps", name=f"kv_ps{i}")
             for i in range(KC)]
    for i in range(KC):
        for cc in range(CC):
            nc.tensor.matmul(
                kv_ps[i], xT_sb[:, cc, i * 128:(i + 1) * 128],
                wkv_sb[:, cc, :],
                start=(cc == 0), stop=(cc == CC - 1))

    # ---------------- topk threshold refinement ----------------
    # invariant: v* (the KSEL-th largest logit) is in (lo, lo + w]
    lo = small.tile([128, 1], F32)
    nc.vector.memset(lo, LO0)
    neg_edges = small.tile([128, 1], F32)
    acc = small.tile([128, 1], F32)
    sel = small.tile([128, 1], F32)
    ssum = small.tile([128, 1], F32)
    sign_scr = pa.tile([128, N], BF16)  # Sign output is never read
    thr_acc = float(2 * KSEL - N)  # acc = #gt - #lt ; acc>=thr <=> #gt>=KSEL
    for r in range(N_ROUNDS):
        wstep = W0 / (128.0 ** (r + 1))
        # neg_edges[p] = -((p+1)*wstep + lo)  computed as iota*(-wstep) - lo
        nc.vector.scalar_tensor_tensor(
            neg_edges, iota128, -wstep, lo, op0=ALU.mult, op1=ALU.subtract)
        nc.scalar.activation(sign_scr, lrep, AF.Sign, bias=neg_edges,
                             scale=1.0, accum_out=acc)
        nc.vector.tensor_single_scalar(sel, acc, thr_acc, op=ALU.is_ge)
        nc.gpsimd.partition_all_reduce(ssum, sel, channels=128,
                                       reduce_op=bass_isa.ReduceOp.add)
        # lo += ssum * wstep   (bit-identical to the edge it selects)
        nc.vector.scalar_tensor_tensor(
            lo, ssum, wstep, lo, op0=ALU.mult, op1=ALU.add)

    # m = (logit > lo) * sigmoid(logit)   per token, replicated on partitions
    grep = pa.tile([128, N], F32)
    nc.scalar.activation(grep, lrep, AF.Sigmoid)
    m_rep = pm.tile([128, N], F32)
    nc.vector.scalar_tensor_tensor(
        m_rep, lrep, lo, grep, op0=ALU.is_gt, op1=ALU.mult)

    # m in token-major layout for K/V row scaling: m_v[p, i] = m[i*128 + p]
    m_dram = dram.tile([N], F32)
    nc.sync.dma_start(m_dram, m_rep[0:1, :])
    m_v = small.tile([128, KC], F32)
    nc.sync.dma_start(m_v, m_dram.rearrange("(i p) -> p i", p=128))

    # ---------------- compacted key index list ----------------
    # sel16[r, c] = token id (16c + r) if selected (m > 0) else -1, in the
    # gpsimd [16, F] free-major wrap; 8 trailing cols hold 128 sentinel
    # entries with id N, whose K|V dram row is all zeros.  sparse_gather
    # drops the negatives: entries 0..KSEL-1 = selected token ids (ascending)
    # followed by the sentinels; the first NSEL entries become the gather
    # index list (KSEL real + 103 zero-row pads).
    m16 = small.tile([16, N // 16 + 8], F32)
    nc.sync.dma_start(m16[:, 0:N // 16], m_dram.rearrange("(c r) -> r c", r=16))
    sel16 = small.tile([16, N // 16 + 8], F32)
    nc.vector.tensor_single_scalar(m16[:, 0:N // 16], m16[:, 0:N // 16], 0.0,
                                   op=ALU.is_gt)
    nc.vector.tensor_tensor(sel16[:, 0:N // 16], m16[:, 0:N // 16], iota16p1,
                            op=ALU.mult)
    nc.vector.tensor_single_scalar(sel16[:, 0:N // 16], sel16[:, 0:N // 16],
                                   -1.0, op=ALU.add)
    nc.vector.memset(sel16[:, N // 16:], float(SENT))
    # output is oversized (1664 slots) so the compaction never overflows it;
    # only the first NSEL entries (cols 0..95) are consumed.
    idxf = small.tile([16, 104], F32)
    nfound = small.tile([1, 1], U32)
    nc.gpsimd.sparse_gather(idxf, sel16, num_found=nfound)
    idx16 = small.tile([16, NSEL // 16], I16)
    nc.vector.tensor_copy(idx16, idxf[:, 0:NSEL // 16])
    # replicate to all 8 gpsimd cores' 16-partition windows
    idx128 = small.tile([128, NSEL // 16], I16)
    for k in range(8):
        nc.sync.dma_start(idx128[16 * k:16 * (k + 1), :], idx16)

    # ---------------- K|V eviction + dram staging ----------------
    # kv row t (bf16): [ K: 4 kv-heads x 64 | V: 4 kv-heads x 64 ], scaled by
    # m[t]; one contiguous write to kvdr [N+1, 512], row N kept all-zero.
    kvdr = dram.tile([N + 1, 2 * KV_D], BF16)
    kv_sb = pkv.tile([128, KC, 2 * KV_D], BF16)
    for i in range(KC):
        nc.vector.tensor_scalar(
            kv_sb[:, i, :], kv_ps[i], m_v[:, i:i + 1], None, op0=ALU.mult)
    # split write: the index list is ascending, so gather call 1 (slots
    # 0..767 = the 768 smallest selected ids, max ~768/0.7+margin) only reads
    # rows < 1408 and can start while the tail chunks are still landing.
    KSPLIT = 11  # token chunks covered by write A (rows 0..1407)
    nc.sync.dma_start(
        kvdr[0:KSPLIT * 128, :].rearrange("(i p) d -> p i d", p=128),
        kv_sb[:, 0:KSPLIT, :])
    nc.sync.dma_start(
        kvdr[KSPLIT * 128:N, :].rearrange("(i p) d -> p i d", p=128),
        kv_sb[:, KSPLIT:, :])
    zrow = pkv.tile([1, 2 * KV_D], BF16)
    nc.vector.memset(zrow, 0.0)
    nc.sync.dma_start(kvdr[N:N + 1, :], zrow)

    # ---------------- gather the selected K|V rows ----------------
    # chunked: the SWDGE descriptor ring holds ~1024 descriptors and a
    # single self-triggered dma_gather cannot reclaim its own entries, so
    # one 1536-index call would crash the device.  Two 768-index calls.
    kv_g = big.tile([128, KCG, 2 * KV_D], BF16)
    for i in range(2):
        isl = idx128[:, i * (GH // 16):(i + 1) * (GH // 16)]
        src = kvdr[0:KSPLIT * 128 + 1, :] if i == 0 else kvdr
        nc.gpsimd.dma_gather(
            kv_g[:, i * (GH // 128):(i + 1) * (GH // 128), :], src, isl,
            GH, GH, 2 * KV_D, transpose=False)

    # ---------------- QT projection (overlaps the gather DMAs) ----------
    # QT[d, q] for my 1024 queries (columns 0..1023 of the rolled xT).
    # Slot layout is permuted so each q-head lands on the same partition range
    # as its GQA kv-head in KT: head h -> slot (h%4)+4*(h//8), partition base
    # ((h//4)%2)*64.  Slot j therefore holds heads (ha, ha+4), ha = j if j<4
    # else j+4, and wq columns are picked per head via a stride-4 head view.
    qt_sb = big.tile([128, H // 2, NQ], BF16)
    for j in range(H // 2):
        for g in range(NQ // 512):
            ps = psum1.tile([128, 512], F32, tag="proj_ps",
                            name=f"q_ps{j}_{g}")
            qs = slice(g * 512, (g + 1) * 512)
            for cc in range(CC):
                nc.tensor.matmul(
                    ps, wq_sb[:, cc, j * 128:(j + 1) * 128],
                    xT_sb[:, cc, qs],
                    start=(cc == 0), stop=(cc == CC - 1))
            nc.vector.tensor_tensor(qt_sb[:, j, qs], ps, m_rep[:, qs], op=ALU.mult)

    # ---------------- KT via PE transpose + V stationary build ----------
    # kt_z[:, hkv] = K of kv-head hkv on its own 64 partitions, zeros on the
    # other 64 -> the logits matmuls contract a full K=128 (the partner
    # q-head's QT rows hit zeros; full-array matmuls keep the PE HAM warm).
    psum_tr = ptr_ctx.enter_context(
        tc.tile_pool(name="psum_tr", bufs=2, space="PSUM"))
    kt_z = big.tile([128, HKV, NSEL], BF16)
    nc.vector.memset(kt_z, 0.0)
    # v65[tok, kc, g, :] = [64 v dims | 1 | 0*63]; the ones column yields the
    # softmax denominator through the same att@V matmul (pad slots included:
    # their p=exp(0)=1 joins the masked keys' constant, hence DENOM_C=512).
    v65 = big.tile([128, KCG, HKV, 128], BF16)
    nc.vector.memset(v65, 0.0)
    # built per gather half so the first attention chunks start early
    for i in range(2):
        csl = slice(i * (GH // 128), (i + 1) * (GH // 128))
        nc.vector.tensor_copy(
            v65[:, csl, :, 0:64],
            kv_g[:, csl, KV_D:].rearrange("p i (g c) -> p i g c", c=64))
        nc.vector.memset(v65[:, csl, :, 64:65], 1.0)
        for kc in range(i * (GH // 128), (i + 1) * (GH // 128)):
            for pairg in range(2):  # kv-heads (0,1) then (2,3)
                trp = psum_tr.tile([128, 128], BF16, tag="tr",
                                   name=f"tr{kc}_{pairg}")
                nc.tensor.transpose(
                    trp, kv_g[:, kc, pairg * 128:(pairg + 1) * 128], idn)
                ksl = slice(kc * 128, (kc + 1) * 128)
                nc.vector.tensor_copy(kt_z[0:64, 2 * pairg, ksl], trp[0:64, :])
                nc.scalar.copy(kt_z[64:128, 2 * pairg + 1, ksl],
                               trp[64:128, :])
    ptr_ctx.close()
    pa_ctx.close()
    pkv_ctx.close()
    pq_ctx.close()
    pm_ctx.close()
    px_ctx.close()  # free xT + phase-1 PSUM

    # ---------------- phase 2: attention over 1536 gathered keys ----------
    ph2_ctx = contextlib.ExitStack()
    ph2 = ph2_ctx.enter_context(tc.tile_pool(name="ph2", bufs=1))
    wo_sb = ph2.tile([128, CC, C], F32R)
    for cc in range(CC):
        nc.sync.dma_start(wo_sb[:, cc, :],
                          wo[cc * 128:(cc + 1) * 128, :].bitcast(F32R))

    patt_ctx = contextlib.ExitStack()
    scr_pool = patt_ctx.enter_context(tc.tile_pool(name="scr_pool", bufs=2))
    p_pool = patt_ctx.enter_context(tc.tile_pool(name="p_pool", bufs=2))
    lg_pool = patt_ctx.enter_context(
        tc.tile_pool(name="lg_pool", bufs=2, space="PSUM"))
    att_pool = patt_ctx.enter_context(
        tc.tile_pool(name="att_pool", bufs=1, space="PSUM"))
    oT_sb = ph2.tile([128, CC, NQ], F32R)
    denom_sb = ph2.tile([16, NQ], F32)

    inv_sqrt_dh = float(1.0 / np.sqrt(DH))
    KQ = 2  # key chunks per P buffer
    # Head pairs (ha, ha+4) sit on disjoint partition halves (row-packed PE).
    pair_heads = [(ha, ha + 4) for ha in (0, 1, 2, 3, 8, 9, 10, 11)]
    for hp, pair in enumerate(pair_heads):
        att_ps = [att_pool.tile([128, NQ], F32, tag=f"att{m}", name=f"att{hp}_{m}")
                  for m in range(2)]
        pend = []  # pipelined attv matmuls: emitted one kc behind logits/exp
        for quarter in range(KCG // KQ):
            p_t = p_pool.tile([128, KQ, N], BF16, tag="p_t", name=f"p_{hp}_{quarter}")
            for kci in range(KQ):
                kc = quarter * KQ + kci
                lg = [lg_pool.tile([128, NQ], F32, tag="lg",
                                   name=f"lg{hp}_{kc}_{m2}") for m2 in range(2)]
                for m in range(2):
                    h = pair[m]
                    hkv = h // 4
                    jq = (h % 4) + 4 * (h // 8)
                    for g in range(NQ // 512):
                        nc.tensor.matmul(
                            lg[m][:, g * 512:(g + 1) * 512],
                            kt_z[:, hkv, kc * 128:(kc + 1) * 128],
                            qt_sb[:, jq, g * 512:(g + 1) * 512],
                            start=True, stop=True)
                for m in range(2):
                    nc.scalar.activation(
                        p_t[:, kci, m * NQ:(m + 1) * NQ], lg[m], AF.Exp,
                        scale=inv_sqrt_dh)
                # emit previous kc's attv matmuls now (keeps PE streaming)
                for f in pend:
                    f()
                pend = []

                def attv(p_t=p_t, kci=kci, kc=kc):
                    for m in range(2):
                        hk = pair[m] // 4
                        for g in range(NQ // 512):
                            nc.tensor.matmul(
                                att_ps[m][:, g * 512:(g + 1) * 512],
                                v65[:, kc, hk, :],
                                p_t[:, kci,
                                    m * NQ + g * 512:m * NQ + (g + 1) * 512],
                                start=(kc == 0), stop=(kc == KCG - 1))

                pend.append(attv)
        for f in pend:
            f()
        # fast evict (releases att psum quickly so PE stays HAM-warm):
        # copy [65, NQ] psum -> sbuf scratch, stash denom row, numerator
        # into oT unscaled; the 1/denom scale happens once after all pairs.
        for m in range(2):
            h = pair[m]
            scr65 = scr_pool.tile([65, NQ], F32R, tag="scr65",
                                  name=f"scr65_{hp}_{m}")
            nc.vector.tensor_copy(scr65, att_ps[m][0:65, :])
            nc.sync.dma_start(denom_sb[h:h + 1, :],
                              scr65[64:65, :].bitcast(F32))
            if h % 2 == 0:
                nc.vector.tensor_copy(oT_sb[0:64, h // 2, :], scr65[0:64, :])
            else:
                # partition shift 0 -> 64 must go through DMA
                nc.sync.dma_start(oT_sb[64:128, h // 2, :], scr65[0:64, :])

    # scale oT rows by 1/(denom + 512): one batched reciprocal (16 lanes),
    # then per-d-chunk broadcast of the two relevant denom rows via a tiny
    # sel8 matmul, and an in-place DVE multiply.  The +512 restores the
    # non-gathered masked keys' exp(0)=1 terms dropped by the compaction.
    rec16 = ph2.tile([16, NQ], F32R)
    rec16_f = ph2.tile([16, NQ], F32)
    nc.vector.tensor_single_scalar(denom_sb, denom_sb, DENOM_C, op=ALU.add)
    with nc.allow_low_precision(reason="2e-5 rel err << output tolerance"):
        nc.vector.reciprocal_approx_fast(out=rec16_f, in_=denom_sb)
    nc.vector.tensor_copy(rec16, rec16_f)
    for dd in range(CC):
        for g in range(NQ // 512):
            bps = lg_pool.tile([128, 512], F32, tag="lg", name=f"bps{dd}_{g}")
            nc.tensor.matmul(
                bps, sel8[:, dd, :], rec16[:, g * 512:(g + 1) * 512],
                start=True, stop=True)
            sl = slice(g * 512, (g + 1) * 512)
            nc.vector.tensor_tensor(
                oT_sb[:, dd, sl], oT_sb[:, dd, sl], bps, op=ALU.mult)
    patt_ctx.close()
    # ---------------- phase 3: output projection ----------------
    ph3_ctx = contextlib.ExitStack()
    psum3 = ph3_ctx.enter_context(tc.tile_pool(name="psum3", bufs=4, space="PSUM"))
    out_pool = ph3_ctx.enter_context(tc.tile_pool(name="out_pool", bufs=2))
    for tt in range(NQ // 128):
        out_sb = out_pool.tile([128, C], F32, tag="out_sb", name=f"out_sb{tt}")
        for og in range(C // 512):
            ps = psum3.tile([128, 512], F32, tag="out_ps", name=f"out_ps{tt}_{og}")
            for dd in range(CC):
                nc.tensor.matmul(
                    ps, oT_sb[:, dd, tt * 128:(tt + 1) * 128],
                    wo_sb[:, dd, og * 512:(og + 1) * 512],
                    start=(dd == 0), stop=(dd == CC - 1))
            nc.scalar.copy(out_sb[:, og * 512:(og + 1) * 512], ps)
        nc.sync.dma_start(out_d[tt * 128:(tt + 1) * 128, :], out_sb)
    ph3_ctx.close()
    ph2_ctx.close()


_NC = None


def build_program():
    global _NC
    if _NC is not None:
        return _NC
    from contextlib import ExitStack

    nc = bacc.Bacc("TRN2", target_bir_lowering=False, debug=False, num_devices=8)
    io = {
        "xT": nc.dram_tensor("xT", (C, N), F32, kind="ExternalInput").ap(),
        "wq": nc.dram_tensor("wq", (C, QT_D), BF16, kind="ExternalInput").ap(),
        "wkv": nc.dram_tensor("wkv", (C, 2 * KV_D), BF16,
                              kind="ExternalInput").ap(),
        "rw": nc.dram_tensor("rw", (C, 1), F32, kind="ExternalInput").ap(),
        "wo": nc.dram_tensor("wo", (C, C), F32, kind="ExternalInput").ap(),
        "sel8": nc.dram_tensor("sel8", (16, CC, 128), F32,
                               kind="ExternalInput").ap(),
        "idn": nc.dram_tensor("idn", (128, 128), BF16,
                              kind="ExternalInput").ap(),
        "out": nc.dram_tensor("out", (NQ, C), F32, kind="ExternalOutput").ap(),
    }
    with TileContext(nc) as tc:
        with ExitStack() as ctx:
            _emit(nc, tc, ctx, io)
    nc.compile()
    _NC = nc
    return nc


def _permute_wq(wq):
    """Column-permute wq so QT slot j's 128 cols = heads (ha, ha+4) contig."""
    wq = np.asarray(wq, np.float32).reshape(C, H, DH)
    order = []
    for j in range(H // 2):
        ha = j if j < 4 else j + 4
        order += [ha, ha + 4]
    return np.ascontiguousarray(wq[:, order, :].reshape(C, H * DH))


def make_in_maps(x, router_w, wq, wk, wv, wo):
    import ml_dtypes

    bf16 = ml_dtypes.bfloat16
    wq = np.ascontiguousarray(_permute_wq(wq).astype(bf16))
    wkv = np.ascontiguousarray(np.concatenate(
        [np.asarray(wk, np.float32), np.asarray(wv, np.float32)],
        axis=1).astype(bf16))
    in_maps = []
    for core in range(8):
        b, h = core // 2, core % 2
        xT_core = np.ascontiguousarray(
            np.roll(np.asarray(x[b], np.float32).T, -h * NQ, axis=1))
        sel8 = np.zeros((16, CC, 128), np.float32)
        for dd in range(CC):
            for p in range(128):
                sel8[2 * dd + p // 64, dd, p] = 1.0
        in_maps.append({
            "xT": xT_core,
            "sel8": sel8,
            "idn": np.eye(128, dtype=bf16),
            "wq": wq,
            "wkv": wkv,
            "rw": np.ascontiguousarray(router_w, dtype=np.float32),
            "wo": np.ascontiguousarray(wo, dtype=np.float32),
        })
    return in_maps


def _numpy_fallback(x, router_w, router_b, wq, bq, wk, bk, wv, bv, wo, bo):
    x = np.asarray(x, np.float32)
    gate = 1.0 / (1.0 + np.exp(-(x @ router_w + router_b)))
    xg = x * gate
    scores = gate[..., 0]
    idx = np.argsort(-scores, axis=-1, kind="stable")[:, :KSEL]
    mask = np.zeros((x.shape[0], x.shape[1]), np.float32)
    np.put_along_axis(mask, idx, 1.0, axis=1)
    xg = xg * mask[..., None]
    q = (xg @ wq + bq).reshape(B, N, H, DH)
    kk = np.repeat((xg @ wk + bk).reshape(B, N, HKV, DH), H // HKV, axis=2)
    v = np.repeat((xg @ wv + bv).reshape(B, N, HKV, DH), H // HKV, axis=2)
    att = np.einsum("bqhd,bkhd->bhqk", q, kk) / np.float32(np.sqrt(DH))
    att = att - att.max(-1, keepdims=True)
    att = np.exp(att)
    att = att / att.sum(-1, keepdims=True)
    o = np.einsum("bhqk,bkhd->bqhd", att, v).reshape(B, N, C)
    return (o @ wo + bo).astype(np.float32)


def kernel(x, router_w, router_b, wq, bq, wk, bk, wv, bv, wo, bo):
    x = np.asarray(x)
    biases = [router_b, bq, bk, bv, bo]
    if any(float(np.abs(np.asarray(t)).max()) != 0.0 for t in biases):
        # The device program folds away the (identically zero) biases; fall
        # back to an exact host implementation if that assumption breaks.
        return _numpy_fallback(x, router_w, router_b, wq, bq, wk, bk, wv, bv,
                               wo, bo)

    from concourse import bass_utils

    nc = build_program()
    in_maps = make_in_maps(x, router_w, wq, wk, wv, wo)
    res = bass_utils.run_bass_kernel_spmd(nc, in_maps, core_ids=list(range(8)))
    out = np.empty((B, N, C), np.float32)
    for core in range(8):
        b, h = core // 2, core % 2
        out[b, h * NQ:(h + 1) * NQ, :] = res.results[core]["out"]
    return out


# revision 34
# speedup vs baseline: 1.2746x; 1.0012x over previous
"""Trainium2 Bass kernel for MIGAttention (topk token masking + GQA attention).

Shapes (hardcoded): B=4, N=2048, C=1024, H=16 heads, HKV=4 kv-heads, DH=64,
keep-ratio 0.7 -> k = 1433 selected tokens per batch row.

Sharding: 8 cores = (batch b in 0..3) x (query-half h in 0..1).  Each core
receives x[b].T with token columns rolled by h*1024 so that its own query
half always occupies columns 0..1023 -> a single SPMD program for all cores.
Each core computes the full gate+topk mask and K/V for all 2048 tokens of its
batch, and attention + output projection for its 1024 queries.

Key compaction: masked tokens have k=v=0, so they contribute exactly
exp(0)=1 to every softmax denominator and 0 to every numerator.  After the
K|V projection the kernel compacts the 1433 selected tokens (padded to 1536
with an all-zero row) via gpsimd sparse_gather (index build) + dma_gather
(packed K|V rows), transposes K on the PE, and runs attention over 12 key
chunks instead of 16.  The gathered pad slots contribute exp(0)=1 each (zero
k row, memset ones column), so the denominator constant is 615-103=512.
"""

import contextlib
import sys

import numpy as np

if "/opt/trn_rl_repo" not in sys.path:
    sys.path.insert(0, "/opt/trn_rl_repo")

import concourse.bass as bass  # noqa: F401
import concourse.bass_isa as bass_isa
import concourse.mybir as mybir
from concourse import bacc
from concourse.tile import TileContext

F32 = mybir.dt.float32
F32R = mybir.dt.float32r
BF16 = mybir.dt.bfloat16
I16 = mybir.dt.int16
I32 = mybir.dt.int32
U32 = mybir.dt.uint32
AF = mybir.ActivationFunctionType
ALU = mybir.AluOpType

B, N, C = 4, 2048, 1024
H, HKV, DH = 16, 4, 64
NQ = N // 2          # queries per core
KSEL = 1433          # max(1, int(N * 0.7))
NSEL = 1536          # gathered key slots (KSEL padded up to mult of 128)
SENT = N             # sentinel index -> all-zero row in the K|V dram buffer
# masked keys each add exp(0)=1 to the softmax denominator; the NSEL-KSEL
# gathered pad slots already contribute theirs (zero k, ones col set)
DENOM_C = float(N - NSEL)
CC = C // 128        # contraction chunks (8)
KC = N // 128        # token chunks (16)
KCG = NSEL // 128    # gathered key chunks (12)
GH = NSEL // 2       # idx per dma_gather call (SWDGE ring holds ~1024 descs)
QT_D = H * DH        # 1024
KV_D = HKV * DH      # 256
N_ROUNDS = 4         # topk threshold refinement rounds (interval 8/128^4
                     # ~3e-8 wide; a logit landing inside is ~4e-5 unlikely,
                     # and an off-by-one selection costs ~0.05% rel err)
LO0, W0 = -4.0, 8.0  # initial logit search interval (logit std ~0.65)


def _emit(nc, tc, ctx, io):
    xT, wq, wkv, rw, wo, out_d = (
        io["xT"], io["wq"], io["wkv"], io["rw"], io["wo"], io["out"])

    # ---------------- long-lived pools ----------------
    # tile pools must close in LIFO order; open order is the exact reverse
    # of close order: psum_r (router psum) closes first, then psum_tr (K
    # transpose psum), pa (router scratch), pkv, pq, pm, px.
    const = ctx.enter_context(tc.tile_pool(name="const", bufs=1))
    small = ctx.enter_context(tc.tile_pool(name="small", bufs=1))
    big = ctx.enter_context(tc.tile_pool(name="big", bufs=1))
    dram = ctx.enter_context(tc.tile_pool(name="dram", bufs=1, space="DRAM"))

    px_ctx = contextlib.ExitStack()   # xT (alive through all projections)
    pm_ctx = contextlib.ExitStack()   # m_rep
    pq_ctx = contextlib.ExitStack()   # wq (Q projection)
    pkv_ctx = contextlib.ExitStack()  # wkv + kv staging
    pa_ctx = contextlib.ExitStack()   # router/refinement scratch
    pr_ctx = contextlib.ExitStack()   # router psum
    ptr_ctx = contextlib.ExitStack()  # K-transpose psum
    px = px_ctx.enter_context(tc.tile_pool(name="px", bufs=1))
    psum1 = px_ctx.enter_context(tc.tile_pool(name="psum1", bufs=6, space="PSUM"))
    pm = pm_ctx.enter_context(tc.tile_pool(name="pm", bufs=1))
    pq = pq_ctx.enter_context(tc.tile_pool(name="pq", bufs=1))
    pkv = pkv_ctx.enter_context(tc.tile_pool(name="pkv", bufs=1))
    pa = pa_ctx.enter_context(tc.tile_pool(name="pa", bufs=1))
    psum_r = pr_ctx.enter_context(tc.tile_pool(name="psum_r", bufs=1, space="PSUM"))

    # ---------------- constants ----------------
    ones_row = const.tile([1, 128], F32)
    nc.vector.memset(ones_row, 1.0)
    iota128_i = const.tile([128, 1], I32)
    nc.gpsimd.iota(iota128_i, pattern=[[0, 1]], base=1, channel_multiplier=1)
    iota128 = const.tile([128, 1], F32)
    nc.vector.tensor_copy(iota128, iota128_i)
    # iota16p1[r, c] = 16*c + r + 1  (token id + 1 in the gpsimd [16, F] wrap)
    iota16_i = const.tile([16, N // 16], I32)
    nc.gpsimd.iota(iota16_i, pattern=[[16, N // 16]], base=1,
                   channel_multiplier=1)
    iota16p1 = const.tile([16, N // 16], F32)
    nc.vector.tensor_copy(iota16p1, iota16_i)
    # attention-phase tiles zero-filled up front on the idle gpsimd queue
    # (kt_z: K with zeroed partner halves; v65: V stationary with ones col)
    kt_z = big.tile([128, HKV, NSEL], BF16)
    nc.gpsimd.memzero(kt_z)
    v65 = big.tile([128, KCG, HKV, 128], BF16)
    nc.gpsimd.memzero(v65)

    # ---------------- router first: logits = x @ rw (full fp32 for exact
    # topk order).  The streamed fp32 x chunks are then converted to the
    # bf16 xT_sb used by the Q and K|V projections (single HBM pass over x).
    rw_sb = pa.tile([128, CC], F32)
    nc.sync.dma_start(rw_sb, rw.rearrange("(cc p) one -> p (cc one)", p=128))
    xT_sb = px.tile([128, CC, N], BF16)
    logits_sb = pa.tile([1, N], F32)
    xr_pool = pa_ctx.enter_context(tc.tile_pool(name="xr_pool", bufs=2))
    # the 4 query-group accumulators pack into 2 psum banks (rows 0 and 64)
    rps = [psum_r.tile([65, 512], F32, tag=f"router_ps{t}",
                       name=f"router_ps{t}") for t in range(2)]

    def rps_row(g):
        return rps[g // 2][(g % 2) * 64:(g % 2) * 64 + 1, :]

    for cc in range(CC):
        xr = xr_pool.tile([128, N], F32, tag="xr", name=f"xr{cc}")
        nc.sync.dma_start(xr, xT[cc * 128:(cc + 1) * 128, :])
        for g in range(4):
            nc.tensor.matmul(
                rps_row(g), rw_sb[:, cc:cc + 1],
                xr[:, g * 512:(g + 1) * 512],
                start=(cc == 0), stop=(cc == CC - 1))
        # bf16 conversion for the projections, alternating engines so the
        # refinement chain (Scalar+Vector) is not starved by either queue
        if cc % 2 == 0:
            nc.scalar.copy(xT_sb[:, cc, :], xr)
        else:
            nc.vector.tensor_copy(xT_sb[:, cc, :], xr)
    for g in range(4):
        nc.vector.tensor_copy(logits_sb[:, g * 512:(g + 1) * 512], rps_row(g))

    # weight loads next on the DMA queues (overlap refinement + projections)
    wkv_sb = pkv.tile([128, CC, 2 * KV_D], BF16)
    for cc in range(CC):
        sl = slice(cc * 128, (cc + 1) * 128)
        nc.sync.dma_start(wkv_sb[:, cc, :], wkv[sl, :])
    wq_sb = pq.tile([128, CC, QT_D], BF16)
    for cc in range(CC):
        nc.sync.dma_start(wq_sb[:, cc, :], wq[cc * 128:(cc + 1) * 128, :])
    # identity (bf16) for PE-transposing the gathered K rows
    idn = const.tile([128, 128], BF16)
    nc.sync.dma_start(idn, io["idn"])
    # sel8[:, dd, p] = one-hot of the head owning oT partition p in d-chunk
    # dd; lhsT for denominator-row -> 128-partition broadcast matmuls
    sel8 = const.tile([16, CC, 128], F32R)
    nc.sync.dma_start(sel8, io["sel8"].bitcast(F32R))

    # replicate logits across all 128 partitions (K=1 matmul broadcast)
    lrep = pa.tile([128, N], F32)
    for g in range(4):
        ps = psum1.tile([128, 512], F32, tag="proj_ps", name=f"bcast{g}")
        nc.tensor.matmul(ps, ones_row, logits_sb[:, g * 512:(g + 1) * 512],
                         start=True, stop=True)
        nc.vector.tensor_copy(lrep[:, g * 512:(g + 1) * 512], ps)
    pr_ctx.close()

    # ---------------- K|V projection matmuls (PE runs these while the
    # scalar/vector/gpsimd engines work through the refinement chain; only
    # the m-scaled evictions below wait for the threshold)
    kv_ps = [psum1.tile([128, 512], F32, tag="proj_ps", name=f"kv_ps{i}")
             for i in range(KC)]
    for i in range(KC):
        for cc in range(CC):
            nc.tensor.matmul(
                kv_ps[i], xT_sb[:, cc, i * 128:(i + 1) * 128],
                wkv_sb[:, cc, :],
                start=(cc == 0), stop=(cc == CC - 1))

    # ---------------- topk threshold refinement ----------------
    # invariant: v* (the KSEL-th largest logit) is in (lo, lo + w]
    lo = small.tile([128, 1], F32)
    nc.vector.memset(lo, LO0)
    neg_edges = small.tile([128, 1], F32)
    acc = small.tile([128, 1], F32)
    sel = small.tile([128, 1], F32)
    ssum = small.tile([128, 1], F32)
    sign_scr = pa.tile([128, N], BF16)  # Sign output is never read
    thr_acc = float(2 * KSEL - N)  # acc = #gt - #lt ; acc>=thr <=> #gt>=KSEL
    for r in range(N_ROUNDS):
        wstep = W0 / (128.0 ** (r + 1))
        # neg_edges[p] = -((p+1)*wstep + lo)  computed as iota*(-wstep) - lo
        nc.vector.scalar_tensor_tensor(
            neg_edges, iota128, -wstep, lo, op0=ALU.mult, op1=ALU.subtract)
        nc.scalar.activation(sign_scr, lrep, AF.Sign, bias=neg_edges,
                             scale=1.0, accum_out=acc)
        nc.vector.tensor_single_scalar(sel, acc, thr_acc, op=ALU.is_ge)
        nc.gpsimd.partition_all_reduce(ssum, sel, channels=128,
                                       reduce_op=bass_isa.ReduceOp.add)
        # lo += ssum * wstep   (bit-identical to the edge it selects)
        nc.vector.scalar_tensor_tensor(
            lo, ssum, wstep, lo, op0=ALU.mult, op1=ALU.add)

    # m = (logit > lo) * sigmoid(logit)   per token, replicated on partitions
    grep = pa.tile([128, N], F32)
    nc.scalar.activation(grep, lrep, AF.Sigmoid)
    m_rep = pm.tile([128, N], F32)
    nc.vector.scalar_tensor_tensor(
        m_rep, lrep, lo, grep, op0=ALU.is_gt, op1=ALU.mult)

    # m in token-major layout for K/V row scaling: m_v[p, i] = m[i*128 + p]
    m_dram = dram.tile([N], F32)
    nc.sync.dma_start(m_dram, m_rep[0:1, :])
    m_v = small.tile([128, KC], F32)
    nc.sync.dma_start(m_v, m_dram.rearrange("(i p) -> p i", p=128))

    # ---------------- compacted key index list ----------------
    # sel16[r, c] = token id (16c + r) if selected (m > 0) else -1, in the
    # gpsimd [16, F] free-major wrap; 8 trailing cols hold 128 sentinel
    # entries with id N, whose K|V dram row is all zeros.  sparse_gather
    # drops the negatives: entries 0..KSEL-1 = selected token ids (ascending)
    # followed by the sentinels; the first NSEL entries become the gather
    # index list (KSEL real + 103 zero-row pads).
    m16 = small.tile([16, N // 16 + 8], F32)
    nc.sync.dma_start(m16[:, 0:N // 16], m_dram.rearrange("(c r) -> r c", r=16))
    sel16 = small.tile([16, N // 16 + 8], F32)
    nc.vector.tensor_single_scalar(m16[:, 0:N // 16], m16[:, 0:N // 16], 0.0,
                                   op=ALU.is_gt)
    nc.vector.tensor_tensor(sel16[:, 0:N // 16], m16[:, 0:N // 16], iota16p1,
                            op=ALU.mult)
    nc.vector.tensor_single_scalar(sel16[:, 0:N // 16], sel16[:, 0:N // 16],
                                   -1.0, op=ALU.add)
    nc.vector.memset(sel16[:, N // 16:], float(SENT))
    # output is oversized (1664 slots) so the compaction never overflows it;
    # only the first NSEL entries (cols 0..95) are consumed.
    idxf = small.tile([16, 104], F32)
    nfound = small.tile([1, 1], U32)
    nc.gpsimd.sparse_gather(idxf, sel16, num_found=nfound)
    idx16 = small.tile([16, NSEL // 16], I16)
    nc.vector.tensor_copy(idx16, idxf[:, 0:NSEL // 16])
    # replicate to all 8 gpsimd cores' 16-partition windows; issued on the
    # gpsimd swdge queue so they don't crawl behind the big HWDGE loads
    idx128 = small.tile([128, NSEL // 16], I16)
    for k in range(8):
        nc.gpsimd.dma_start(idx128[16 * k:16 * (k + 1), :], idx16)

    # ---------------- K|V eviction + dram staging ----------------
    # kv row t (bf16): [ K: 4 kv-heads x 64 | V: 4 kv-heads x 64 ], scaled by
    # m[t]; one contiguous write to kvdr [N+1, 512], row N kept all-zero.
    kvdr = dram.tile([N + 1, 2 * KV_D], BF16)
    kv_sb = pkv.tile([128, KC, 2 * KV_D], BF16)
    for i in range(KC):
        nc.vector.tensor_scalar(
            kv_sb[:, i, :], kv_ps[i], m_v[:, i:i + 1], None, op0=ALU.mult)
    # split write: the index list is ascending, so gather call 1 (slots
    # 0..767 = the 768 smallest selected ids, max ~768/0.7+margin) only reads
    # rows < 1408 and can start while the tail chunks are still landing.
    KSPLIT = 11  # token chunks covered by write A (rows 0..1407)
    nc.gpsimd.dma_start(
        kvdr[0:KSPLIT * 128, :].rearrange("(i p) d -> p i d", p=128),
        kv_sb[:, 0:KSPLIT, :])
    nc.gpsimd.dma_start(
        kvdr[KSPLIT * 128:N, :].rearrange("(i p) d -> p i d", p=128),
        kv_sb[:, KSPLIT:, :])
    zrow = pkv.tile([1, 2 * KV_D], BF16)
    nc.vector.memset(zrow, 0.0)
    nc.gpsimd.dma_start(kvdr[N:N + 1, :], zrow)

    # ---------------- gather the selected K|V rows ----------------
    # chunked: the SWDGE descriptor ring holds ~1024 descriptors and a
    # single self-triggered dma_gather cannot reclaim its own entries, so
    # one 1536-index call would crash the device.  Two 768-index calls.
    kv_g = big.tile([128, KCG, 2 * KV_D], BF16)
    for i in range(2):
        isl = idx128[:, i * (GH // 16):(i + 1) * (GH // 16)]
        src = kvdr[0:KSPLIT * 128 + 1, :] if i == 0 else kvdr
        nc.gpsimd.dma_gather(
            kv_g[:, i * (GH // 128):(i + 1) * (GH // 128), :], src, isl,
            GH, GH, 2 * KV_D, transpose=False)

    # ---------------- QT projection (overlaps the gather DMAs) ----------
    # QT[d, q] for my 1024 queries (columns 0..1023 of the rolled xT).
    # Slot layout is permuted so each q-head lands on the same partition range
    # as its GQA kv-head in KT: head h -> slot (h%4)+4*(h//8), partition base
    # ((h//4)%2)*64.  Slot j therefore holds heads (ha, ha+4), ha = j if j<4
    # else j+4, and wq columns are picked per head via a stride-4 head view.
    qt_sb = big.tile([128, H // 2, NQ], BF16)
    for j in range(H // 2):
        for g in range(NQ // 512):
            ps = psum1.tile([128, 512], F32, tag="proj_ps",
                            name=f"q_ps{j}_{g}")
            qs = slice(g * 512, (g + 1) * 512)
            for cc in range(CC):
                nc.tensor.matmul(
                    ps, wq_sb[:, cc, j * 128:(j + 1) * 128],
                    xT_sb[:, cc, qs],
                    start=(cc == 0), stop=(cc == CC - 1))
            nc.vector.tensor_tensor(qt_sb[:, j, qs], ps, m_rep[:, qs], op=ALU.mult)

    # ---------------- KT via PE transpose + V stationary build ----------
    # kt_z[:, hkv] = K of kv-head hkv on its own 64 partitions, zeros on the
    # other 64 -> the logits matmuls contract a full K=128 (the partner
    # q-head's QT rows hit zeros; full-array matmuls keep the PE HAM warm).
    psum_tr = ptr_ctx.enter_context(
        tc.tile_pool(name="psum_tr", bufs=2, space="PSUM"))
    # v65[tok, kc, g, :] = [64 v dims | 1 | 0*63]; the ones column yields the
    # softmax denominator through the same att@V matmul (pad slots included:
    # their p=exp(0)=1 joins the masked keys' constant, hence DENOM_C=512).
    # Built per gather half so the first attention chunks start early.
    for i in range(2):
        csl = slice(i * (GH // 128), (i + 1) * (GH // 128))
        nc.vector.tensor_copy(
            v65[:, csl, :, 0:64],
            kv_g[:, csl, KV_D:].rearrange("p i (g c) -> p i g c", c=64))
        nc.vector.memset(v65[:, csl, :, 64:65], 1.0)
        for kc in range(i * (GH // 128), (i + 1) * (GH // 128)):
            for pairg in range(2):  # kv-heads (0,1) then (2,3)
                trp = psum_tr.tile([128, 128], BF16, tag="tr",
                                   name=f"tr{kc}_{pairg}")
                nc.tensor.transpose(
                    trp, kv_g[:, kc, pairg * 128:(pairg + 1) * 128], idn)
                ksl = slice(kc * 128, (kc + 1) * 128)
                nc.vector.tensor_copy(kt_z[0:64, 2 * pairg, ksl], trp[0:64, :])
                nc.scalar.copy(kt_z[64:128, 2 * pairg + 1, ksl],
                               trp[64:128, :])
    ptr_ctx.close()
    pa_ctx.close()
    pkv_ctx.close()
    pq_ctx.close()
    pm_ctx.close()
    px_ctx.close()  # free xT + phase-1 PSUM

    # ---------------- phase 2: attention over 1536 gathered keys ----------
    ph2_ctx = contextlib.ExitStack()
    ph2 = ph2_ctx.enter_context(tc.tile_pool(name="ph2", bufs=1))
    wo_sb = ph2.tile([128, CC, C], F32R)  # loaded mid-attention, see below

    patt_ctx = contextlib.ExitStack()
    scr_pool = patt_ctx.enter_context(tc.tile_pool(name="scr_pool", bufs=2))
    p_pool = patt_ctx.enter_context(tc.tile_pool(name="p_pool", bufs=2))
    lg_pool = patt_ctx.enter_context(
        tc.tile_pool(name="lg_pool", bufs=2, space="PSUM"))
    att_pool = patt_ctx.enter_context(
        tc.tile_pool(name="att_pool", bufs=1, space="PSUM"))
    oT_sb = ph2.tile([128, CC, NQ], F32R)
    denom_sb = ph2.tile([16, NQ], F32)

    inv_sqrt_dh = float(1.0 / np.sqrt(DH))
    KQ = 2  # key chunks per P buffer
    # Head pairs (ha, ha+4) sit on disjoint partition halves (row-packed PE).
    pair_heads = [(ha, ha + 4) for ha in (0, 1, 2, 3, 8, 9, 10, 11)]
    for hp, pair in enumerate(pair_heads):
        if hp == 2:
            # wo loads issued here: the head-phase DMA backlog has drained
            # and the transfer still finishes long before the projection
            for cc in range(CC):
                nc.sync.dma_start(wo_sb[:, cc, :],
                                  wo[cc * 128:(cc + 1) * 128, :].bitcast(F32R))
        att_ps = [att_pool.tile([128, NQ], F32, tag=f"att{m}", name=f"att{hp}_{m}")
                  for m in range(2)]
        pend = []  # pipelined attv matmuls: emitted one kc behind logits/exp
        for quarter in range(KCG // KQ):
            p_t = p_pool.tile([128, KQ, N], BF16, tag="p_t", name=f"p_{hp}_{quarter}")
            for kci in range(KQ):
                kc = quarter * KQ + kci
                lg = [lg_pool.tile([128, NQ], F32, tag="lg",
                                   name=f"lg{hp}_{kc}_{m2}") for m2 in range(2)]
                for m in range(2):
                    h = pair[m]
                    hkv = h // 4
                    jq = (h % 4) + 4 * (h // 8)
                    for g in range(NQ // 512):
                        nc.tensor.matmul(
                            lg[m][:, g * 512:(g + 1) * 512],
                            kt_z[:, hkv, kc * 128:(kc + 1) * 128],
                            qt_sb[:, jq, g * 512:(g + 1) * 512],
                            start=True, stop=True)
                for m in range(2):
                    nc.scalar.activation(
                        p_t[:, kci, m * NQ:(m + 1) * NQ], lg[m], AF.Exp,
                        scale=inv_sqrt_dh)
                # emit previous kc's attv matmuls now (keeps PE streaming)
                for f in pend:
                    f()
                pend = []

                def attv(p_t=p_t, kci=kci, kc=kc):
                    for m in range(2):
                        hk = pair[m] // 4
                        for g in range(NQ // 512):
                            nc.tensor.matmul(
                                att_ps[m][:, g * 512:(g + 1) * 512],
                                v65[:, kc, hk, :],
                                p_t[:, kci,
                                    m * NQ + g * 512:m * NQ + (g + 1) * 512],
                                start=(kc == 0), stop=(kc == KCG - 1))

                pend.append(attv)
        for f in pend:
            f()
        # fast evict (releases att psum quickly so PE stays HAM-warm):
        # copy [65, NQ] psum -> sbuf scratch, stash denom row, numerator
        # into oT unscaled; the 1/denom scale happens once after all pairs.
        for m in range(2):
            h = pair[m]
            scr65 = scr_pool.tile([65, NQ], F32R, tag="scr65",
                                  name=f"scr65_{hp}_{m}")
            nc.vector.tensor_copy(scr65, att_ps[m][0:65, :])
            nc.sync.dma_start(denom_sb[h:h + 1, :],
                              scr65[64:65, :].bitcast(F32))
            if h % 2 == 0:
                nc.vector.tensor_copy(oT_sb[0:64, h // 2, :], scr65[0:64, :])
            else:
                # partition shift 0 -> 64 must go through DMA
                nc.sync.dma_start(oT_sb[64:128, h // 2, :], scr65[0:64, :])

    # scale oT rows by 1/(denom + 512): one batched reciprocal (16 lanes),
    # then per-d-chunk broadcast of the two relevant denom rows via a tiny
    # sel8 matmul, and an in-place DVE multiply.  The +512 restores the
    # non-gathered masked keys' exp(0)=1 terms dropped by the compaction.
    rec16 = ph2.tile([16, NQ], F32R)
    rec16_f = ph2.tile([16, NQ], F32)
    nc.vector.tensor_single_scalar(denom_sb, denom_sb, DENOM_C, op=ALU.add)
    with nc.allow_low_precision(reason="2e-5 rel err << output tolerance"):
        nc.vector.reciprocal_approx_fast(out=rec16_f, in_=denom_sb)
    nc.vector.tensor_copy(rec16, rec16_f)
    for dd in range(CC):
        for g in range(NQ // 512):
            bps = lg_pool.tile([128, 512], F32, tag="lg", name=f"bps{dd}_{g}")
            nc.tensor.matmul(
                bps, sel8[:, dd, :], rec16[:, g * 512:(g + 1) * 512],
                start=True, stop=True)
            sl = slice(g * 512, (g + 1) * 512)
            nc.vector.tensor_tensor(
                oT_sb[:, dd, sl], oT_sb[:, dd, sl], bps, op=ALU.mult)
    patt_ctx.close()
    # ---------------- phase 3: output projection ----------------
    ph3_ctx = contextlib.ExitStack()
    psum3 = ph3_ctx.enter_context(tc.tile_pool(name="psum3", bufs=4, space="PSUM"))
    out_pool = ph3_ctx.enter_context(tc.tile_pool(name="out_pool", bufs=2))
    for tt in range(NQ // 128):
        out_sb = out_pool.tile([128, C], F32, tag="out_sb", name=f"out_sb{tt}")
        for og in range(C // 512):
            ps = psum3.tile([128, 512], F32, tag="out_ps", name=f"out_ps{tt}_{og}")
            for dd in range(CC):
                nc.tensor.matmul(
                    ps, oT_sb[:, dd, tt * 128:(tt + 1) * 128],
                    wo_sb[:, dd, og * 512:(og + 1) * 512],
                    start=(dd == 0), stop=(dd == CC - 1))
            nc.scalar.copy(out_sb[:, og * 512:(og + 1) * 512], ps)
        nc.sync.dma_start(out_d[tt * 128:(tt + 1) * 128, :], out_sb)
    ph3_ctx.close()
    ph2_ctx.close()


_NC = None


def build_program():
    global _NC
    if _NC is not None:
        return _NC
    from contextlib import ExitStack

    nc = bacc.Bacc("TRN2", target_bir_lowering=False, debug=False, num_devices=8)
    io = {
        "xT": nc.dram_tensor("xT", (C, N), F32, kind="ExternalInput").ap(),
        "wq": nc.dram_tensor("wq", (C, QT_D), BF16, kind="ExternalInput").ap(),
        "wkv": nc.dram_tensor("wkv", (C, 2 * KV_D), BF16,
                              kind="ExternalInput").ap(),
        "rw": nc.dram_tensor("rw", (C, 1), F32, kind="ExternalInput").ap(),
        "wo": nc.dram_tensor("wo", (C, C), F32, kind="ExternalInput").ap(),
        "sel8": nc.dram_tensor("sel8", (16, CC, 128), F32,
                               kind="ExternalInput").ap(),
        "idn": nc.dram_tensor("idn", (128, 128), BF16,
                              kind="ExternalInput").ap(),
        "out": nc.dram_tensor("out", (NQ, C), F32, kind="ExternalOutput").ap(),
    }
    with TileContext(nc) as tc:
        with ExitStack() as ctx:
            _emit(nc, tc, ctx, io)
    nc.compile()
    _NC = nc
    return nc


def _permute_wq(wq):
    """Column-permute wq so QT slot j's 128 cols = heads (ha, ha+4) contig."""
    wq = np.asarray(wq, np.float32).reshape(C, H, DH)
    order = []
    for j in range(H // 2):
        ha = j if j < 4 else j + 4
        order += [ha, ha + 4]
    return np.ascontiguousarray(wq[:, order, :].reshape(C, H * DH))


def make_in_maps(x, router_w, wq, wk, wv, wo):
    import ml_dtypes

    bf16 = ml_dtypes.bfloat16
    wq = np.ascontiguousarray(_permute_wq(wq).astype(bf16))
    wkv = np.ascontiguousarray(np.concatenate(
        [np.asarray(wk, np.float32), np.asarray(wv, np.float32)],
        axis=1).astype(bf16))
    in_maps = []
    for core in range(8):
        b, h = core // 2, core % 2
        xT_core = np.ascontiguousarray(
            np.roll(np.asarray(x[b], np.float32).T, -h * NQ, axis=1))
        sel8 = np.zeros((16, CC, 128), np.float32)
        for dd in range(CC):
            for p in range(128):
                sel8[2 * dd + p // 64, dd, p] = 1.0
        in_maps.append({
            "xT": xT_core,
            "sel8": sel8,
            "idn": np.eye(128, dtype=bf16),
            "wq": wq,
            "wkv": wkv,
            "rw": np.ascontiguousarray(router_w, dtype=np.float32),
            "wo": np.ascontiguousarray(wo, dtype=np.float32),
        })
    return in_maps


def _numpy_fallback(x, router_w, router_b, wq, bq, wk, bk, wv, bv, wo, bo):
    x = np.asarray(x, np.float32)
    gate = 1.0 / (1.0 + np.exp(-(x @ router_w + router_b)))
    xg = x * gate
    scores = gate[..., 0]
    idx = np.argsort(-scores, axis=-1, kind="stable")[:, :KSEL]
    mask = np.zeros((x.shape[0], x.shape[1]), np.float32)
    np.put_along_axis(mask, idx, 1.0, axis=1)
    xg = xg * mask[..., None]
    q = (xg @ wq + bq).reshape(B, N, H, DH)
    kk = np.repeat((xg @ wk + bk).reshape(B, N, HKV, DH), H // HKV, axis=2)
    v = np.repeat((xg @ wv + bv).reshape(B, N, HKV, DH), H // HKV, axis=2)
    att = np.einsum("bqhd,bkhd->bhqk", q, kk) / np.float32(np.sqrt(DH))
    att = att - att.max(-1, keepdims=True)
    att = np.exp(att)
    att = att / att.sum(-1, keepdims=True)
    o = np.einsum("bhqk,bkhd->bqhd", att, v).reshape(B, N, C)
    return (o @ wo + bo).astype(np.float32)


def kernel(x, router_w, router_b, wq, bq, wk, bk, wv, bv, wo, bo):
    x = np.asarray(x)
    biases = [router_b, bq, bk, bv, bo]
    if any(float(np.abs(np.asarray(t)).max()) != 0.0 for t in biases):
        # The device program folds away the (identically zero) biases; fall
        # back to an exact host implementation if that assumption breaks.
        return _numpy_fallback(x, router_w, router_b, wq, bq, wk, bk, wv, bv,
                               wo, bo)

    from concourse import bass_utils

    nc = build_program()
    in_maps = make_in_maps(x, router_w, wq, wk, wv, wo)
    res = bass_utils.run_bass_kernel_spmd(nc, in_maps, core_ids=list(range(8)))
    out = np.empty((B, N, C), np.float32)
    for core in range(8):
        b, h = core // 2, core % 2
        out[b, h * NQ:(h + 1) * NQ, :] = res.results[core]["out"]
    return out
